# revision 1
# baseline (speedup 1.0000x reference)
"""CrossBatchAttention Trainium2 kernel — 8-core tensor-parallel SPMD.

Layout strategy: every on-chip tensor is kept in transposed [feature, batch]
layout so the TensorEngine contraction dim is always on partitions and no
on-chip transposes are needed. Host numpy does all transposes / casts /
shard slicing, and adds the residual hidden_states at the end.

Per core c (of 8):
  phase 1: QT/KT [512,2048], V [2048,512] (4 local heads), g1X (gate W1
           X-part, gh-shard) — from X^T streamed in batch-quarters.
  phase 2: per (head, batch-quarter): S^T = K^T@Q^T per j-tile, ACT
           Exp(scale*s + mask_bias) straight out of PSUM, diagonal zeroed
           with a (1-I) multiply, denominator via all-ones lhsT matmul
           (row-broadcast for free), O^T = V@P^T, normalize with
           reciprocal_approx_fast. AllGather O^T per head.
  phase 3: cross^T[hid-shard] = Wo[:, shard]^T @ OT_full (column-parallel,
           no reduce), k-grouped by AG chunk; the last group runs
           i-chunk-major and feeds the gate chain per chunk.
  phase 4 (pipelined per i-chunk inside phase 3's last group):
           g1C partial -> ReduceScatter(gh) -> gelu -> AllGather(g^T) ->
           logits[hid-shard] = gW2[:, shard]^T @ gT -> sigmoid ->
           out^T = gate * cross^T.
Host: concat 8 [512,2048] shards, transpose, add X -> [2048,4096] f32.
"""

import numpy as np
import ml_dtypes

import concourse.bass as bass
import concourse.mybir as mybir
import concourse.tile as tile
from concourse import bacc
from concourse import bass_utils

BF16 = mybir.dt.bfloat16
F32 = mybir.dt.float32
F8 = mybir.dt.float8e4
WO_SCALE = 64.0

B = 2048
HID = 4096
NH = 32
HD = 128
GH = 1024
NC_ = 8
HPC = NH // NC_          # heads per core = 4
HS = HID // NC_          # hid shard = 512
GS = GH // NC_           # gate-hidden shard = 128
SCALE = 1.0 / float(np.sqrt(HD))

KT_TILES = HID // 128    # 32 k-tiles over the 4096 contraction
JT = B // 128            # 16 j-tiles over keys
IC = B // 512            # 4 i-chunks of 512 over batch

# CoreSim doesn't implement Gelu; debug_sim swaps in Tanh.
GELU_FUNC = mybir.ActivationFunctionType.Gelu


def _build_program():
    nc = bacc.Bacc(
        "TRN2",
        target_bir_lowering=False,
        debug=False,
        enable_asserts=False,
        num_devices=NC_,
    )

    # ---- I/O declarations (per-core shapes) ----
    xt_bf = nc.dram_tensor("xt_bf", [HID, B], BF16, kind="ExternalInput").ap()
    wq_d = nc.dram_tensor("wq", [HID, HS], BF16, kind="ExternalInput").ap()
    wk_d = nc.dram_tensor("wk", [HID, HS], BF16, kind="ExternalInput").ap()
    wv_d = nc.dram_tensor("wv", [HID, HS], BF16, kind="ExternalInput").ap()
    wo_d = nc.dram_tensor("wo", [HID, HS], F8, kind="ExternalInput").ap()
    gw1x_d = nc.dram_tensor("gw1x", [HID, GS], BF16, kind="ExternalInput").ap()
    gw1c_d = nc.dram_tensor("gw1c", [HS, GH], BF16, kind="ExternalInput").ap()
    gw2_d = nc.dram_tensor("gw2", [GH, HS], BF16, kind="ExternalInput").ap()
    gb1_d = nc.dram_tensor("gb1", [GS, 1], F32, kind="ExternalInput").ap()
    gb2_d = nc.dram_tensor("gb2", [128, 4], F32, kind="ExternalInput").ap()
    maskb_d = nc.dram_tensor("maskb", [128, JT], F32, kind="ExternalInput").ap()
    diagm_d = nc.dram_tensor("diagm", [128, 128], BF16, kind="ExternalInput").ap()
    out_d = nc.dram_tensor("out", [HS, B], F32, kind="ExternalOutput").ap()

    groups = [list(range(NC_))]

    with tile.TileContext(nc) as tc:
        with (
            tc.tile_pool(name="persist", bufs=1) as persist,
            tc.tile_pool(name="psum", bufs=1, space="PSUM") as psum,
            tc.tile_pool(name="dram", bufs=1, space="DRAM") as dram,
        ):
            # ---------- persistent SBUF ----------
            qt_sb = persist.tile([128, HPC, B], BF16)     # [d, head, i] 2MB
            kt_sb = persist.tile([128, HPC, B], BF16)     # 2MB
            v_sb = persist.tile([128, JT, HS], BF16)      # [j_in, j_tile, hd] 2MB
            g1x_sb = persist.tile([128, B], F32)          # gate W1 X-part 1MB
            maskb_sb = persist.tile([128, JT], F32)
            diagm_sb = persist.tile([128, 128], BF16)
            ones_sb = persist.tile([128, 128], BF16)
            gb1_sb = persist.tile([GS, 1], F32)
            gb2_sb = persist.tile([128, 4], F32)

            nc.sync.dma_start(out=maskb_sb, in_=maskb_d)
            nc.sync.dma_start(out=diagm_sb, in_=diagm_d)
            nc.sync.dma_start(out=gb1_sb, in_=gb1_d)
            nc.sync.dma_start(out=gb2_sb, in_=gb2_d)
            nc.vector.memset(ones_sb, 1.0)

            # ---------- DRAM bounce buffers for collectives ----------
            # O^T AllGather in per-(head, batch-half) chunks: last chunk
            # lands earlier so the out_proj tail starts sooner.
            ag_in = dram.tile([HPC, 2, 128, B // 2], F8)
            ag_out = [[None, None] for _ in range(HPC)]
            for h in range(HPC):
                for hf in range(2):
                    t_ag = dram.tile(
                        [NC_ * 128, B // 2], F8, addr_space="Shared",
                        name=f"ag_out{h}_{hf}"
                    )
                    ag_out[h][hf] = t_ag
            rs_in_c, rs_out_c, ag2_in_c, ag2_out_c = [], [], [], []
            for icc in range(IC):
                t_ri = dram.tile([GH, 512], BF16, name=f"rs_in{icc}")
                t_ro = dram.tile([GS, 512], BF16, name=f"rs_out{icc}")
                t_ai = dram.tile([GS, 512], BF16, name=f"ag2_in{icc}")
                t_ao = dram.tile([GH, 512], BF16, addr_space="Shared",
                                 name=f"ag2_out{icc}")
                rs_in_c.append(t_ri)
                rs_out_c.append(t_ro)
                ag2_in_c.append(t_ai)
                ag2_out_c.append(t_ao)

            warm_rs_i = dram.tile([GH, 64], BF16)
            warm_rs_o = dram.tile([GS, 64], BF16)
            warm_ag_i = dram.tile([GS, 64], BF16)
            warm_ag_o = dram.tile([GH, 64], BF16, addr_space="Shared")
            nc.gpsimd.collective_compute(
                "ReduceScatter", mybir.AluOpType.add, replica_groups=groups,
                ins=[warm_rs_i.opt()], outs=[warm_rs_o.opt()],
            )
            nc.gpsimd.collective_compute(
                "AllGather", mybir.AluOpType.bypass, replica_groups=groups,
                ins=[warm_ag_i.opt()], outs=[warm_ag_o.opt()],
            )

            # =====================================================
            # Phase 1: projections, streamed in batch-quarters
            # =====================================================
            with tc.tile_pool(name="p1", bufs=1) as p1:
                gw1x_sb = p1.tile([128, KT_TILES, GS], BF16, tag="gw1x", bufs=1)
                for q in range(IC):  # 4 quarters of 512 batch elems
                    isl = slice(q * 512, (q + 1) * 512)
                    xt_q = p1.tile([128, KT_TILES, 512], BF16, tag="xt", bufs=2)
                    # chunked DMA so the first matmuls start early
                    for kk in range(4):
                        nc.sync.dma_start(
                            out=xt_q[:, kk * 8:(kk + 1) * 8, :],
                            in_=xt_bf[kk * 1024:(kk + 1) * 1024, isl].rearrange(
                                "(t p) i -> p t i", p=128
                            ),
                        )

                    def load_w_chunks(wd, nm):
                        chunks = []
                        for hh in range(4):
                            w_sb = p1.tile([128, 8, HS], BF16,
                                           tag="w", bufs=6, name=nm + str(hh))
                            nc.sync.dma_start(
                                out=w_sb,
                                in_=wd[hh * 1024:(hh + 1) * 1024, :].rearrange(
                                    "(t p) m -> p t m", p=128
                                ),
                            )
                            chunks.append(w_sb)
                        return chunks

                    def w_slice(chunks, k, msl):
                        return chunks[k // 8][:, k % 8, msl]

                    for wd, dst, nm in ((wq_d, qt_sb, "wq"), (wk_d, kt_sb, "wk")):
                        wh = load_w_chunks(wd, nm)
                        if q == 0 and nm == "wq":
                            nc.sync.dma_start(
                                out=gw1x_sb,
                                in_=gw1x_d.rearrange("(t p) m -> p t m", p=128),
                            )
                        for m in range(4):
                            ps = psum.tile([128, 512], F32, tag="mm", bufs=3,
                                           name="ps_pr")
                            for k in range(KT_TILES):
                                nc.tensor.matmul(
                                    ps,
                                    lhsT=w_slice(wh, k,
                                                 slice(m * 128, (m + 1) * 128)),
                                    rhs=xt_q[:, k, :],
                                    start=(k == 0),
                                    stop=(k == KT_TILES - 1),
                                )
                            nc.vector.tensor_copy(dst[:, m, isl], ps)
                    # V in natural [j, d] layout: lhsT = X^T tiles
                    wvh = load_w_chunks(wv_d, "wv")
                    for it in range(4):  # 4 i-tiles of 128 in this quarter
                        ps = psum.tile([128, 512], F32, tag="mm", bufs=3,
                                       name="ps_v")
                        for k in range(KT_TILES):
                            nc.tensor.matmul(
                                ps,
                                lhsT=xt_q[:, k, it * 128:(it + 1) * 128],
                                rhs=w_slice(wvh, k, slice(0, HS)),
                                start=(k == 0),
                                stop=(k == KT_TILES - 1),
                            )
                        nc.vector.tensor_copy(v_sb[:, q * 4 + it, :], ps)
                    # gate W1 X-part (gh-shard output)
                    ps = psum.tile([128, 512], F32, tag="mm", bufs=3, name="ps_g1x")
                    for k in range(KT_TILES):
                        nc.tensor.matmul(
                            ps,
                            lhsT=gw1x_sb[:, k, :],
                            rhs=xt_q[:, k, :],
                            start=(k == 0),
                            stop=(k == KT_TILES - 1),
                        )
                    nc.vector.tensor_copy(g1x_sb[:, isl], ps)

            # =====================================================
            # Phase 2: attention per (head, batch-quarter)
            # =====================================================
            with tc.tile_pool(name="p2", bufs=1) as p2:
                for h in range(HPC):
                    for q in range(IC):
                        qsl = slice(q * 512, (q + 1) * 512)
                        den_ps = psum.tile([128, 512], F32, tag="den", bufs=2)
                        ot_ps = psum.tile([128, 512], F32, tag="ot", bufs=2)
                        pt = p2.tile([128, JT, 512], BF16, tag="pt", bufs=2)
                        for j in range(JT):
                            st = psum.tile([128, 512], F32, tag="mm", bufs=3,
                                           name="st")
                            nc.tensor.matmul(
                                st,
                                lhsT=kt_sb[:, h, j * 128:(j + 1) * 128],
                                rhs=qt_sb[:, h, qsl],
                                start=True,
                                stop=True,
                            )
                            nc.scalar.activation(
                                pt[:, j, :],
                                st,
                                mybir.ActivationFunctionType.Exp,
                                bias=maskb_sb[:, j:j + 1],
                                scale=SCALE,
                            )
                            # zero the self-attention diagonal block
                            if j // 4 == q:
                                c0 = (j % 4) * 128
                                nc.vector.tensor_mul(
                                    pt[:, j, c0:c0 + 128],
                                    pt[:, j, c0:c0 + 128],
                                    diagm_sb,
                                )
                        for j in range(JT):
                            nc.tensor.matmul(
                                den_ps,
                                lhsT=ones_sb,
                                rhs=pt[:, j, :],
                                start=(j == 0),
                                stop=(j == JT - 1),
                            )
                            nc.tensor.matmul(
                                ot_ps,
                                lhsT=v_sb[:, j, h * 128:(h + 1) * 128],
                                rhs=pt[:, j, :],
                                start=(j == 0),
                                stop=(j == JT - 1),
                            )
                        rec = p2.tile([128, 512], F32, tag="rec", bufs=2)
                        nc.vector.reciprocal_approx_fast(out=rec, in_=den_ps)
                        otc = p2.tile([128, 512], F8, tag="otc", bufs=2)
                        nc.vector.tensor_mul(otc, ot_ps, rec)
                        nc.sync.dma_start(
                            out=ag_in[h, q // 2, :, (q % 2) * 512:
                                      (q % 2) * 512 + 512],
                            in_=otc,
                        )
                        if q % 2 == 1:
                            hf = q // 2
                            nc.gpsimd.collective_compute(
                                "AllGather",
                                mybir.AluOpType.bypass,
                                replica_groups=groups,
                                ins=[ag_in[h, hf].opt()],
                                outs=[ag_out[h][hf].opt()],
                            )

            # =====================================================
            # Phase 3 + 4: out_proj (k-grouped by AG chunk); the last
            # group is i-chunk-major and drives the gate-MLP pipeline
            # =====================================================
            with tc.tile_pool(name="p34", bufs=1) as p34:
                cacc = p34.tile([128, 4, B], BF16, tag="cacc", bufs=1)
                wo_sb = p34.tile([128, KT_TILES, HS], F8, tag="wo", bufs=1)
                nc.sync.dma_start(
                    out=wo_sb, in_=wo_d.rearrange("(t p) m -> p t m", p=128)
                )
                gw1c_sb = p34.tile([128, 4, GH], BF16, tag="gw1c", bufs=1)
                nc.sync.dma_start(
                    out=gw1c_sb, in_=gw1c_d.rearrange("(t p) m -> p t m", p=128)
                )
                gw2_sb = p34.tile([128, NC_, HS], BF16, tag="gw2", bufs=1)
                nc.sync.dma_start(
                    out=gw2_sb, in_=gw2_d.rearrange("(t p) m -> p t m", p=128)
                )
                g1c_sb = p34.tile([128, B], BF16, tag="g1c", bufs=1)

                def outproj_group(t, ic):
                    csl = slice(ic * 512, (ic + 1) * 512)
                    otg = p34.tile([128, NC_, 512], F8, tag="otg", bufs=4,
                                   name="otg")
                    nc.sync.dma_start(
                        out=otg,
                        in_=ag_out[t][ic // 2][:, (ic % 2) * 512:
                                               (ic % 2) * 512 + 512].rearrange(
                            "(r p) i -> p r i", p=128
                        ),
                    )
                    for m in range(4):
                        ps = psum.tile([128, 512], F32, tag="mm", bufs=3,
                                       name="ps_wo")
                        for r in range(NC_):
                            nc.tensor.matmul(
                                ps,
                                lhsT=wo_sb[:, t * NC_ + r,
                                           m * 128:(m + 1) * 128],
                                rhs=otg[:, r, :],
                                start=(r == 0),
                                stop=(r == NC_ - 1),
                            )
                        if t == 0:
                            nc.vector.tensor_scalar_mul(
                                cacc[:, m, csl], ps, 1.0 / WO_SCALE
                            )
                        else:
                            nc.vector.scalar_tensor_tensor(
                                cacc[:, m, csl], ps, 1.0 / WO_SCALE,
                                cacc[:, m, csl],
                                op0=mybir.AluOpType.mult,
                                op1=mybir.AluOpType.add,
                            )

                for t in range(HPC - 1):
                    for ic in range(IC):
                        outproj_group(t, ic)

                # ---- last k-group, i-chunk-major, feeding the gate chain.
                # Pass 1: all PE compute + collective issues. CC-dependent
                # loads/adds go on the gpsimd queue so neither the PE nor the
                # sync-DMA queue ever waits on a collective.
                gtf_tiles = []
                for ic in range(IC):
                    csl = slice(ic * 512, (ic + 1) * 512)
                    outproj_group(HPC - 1, ic)
                    for gm in range(NC_):  # 8 gh-tiles of g1C partial
                        ps = psum.tile([128, 512], F32, tag="mm", bufs=3,
                                       name="ps_g1c")
                        for r in range(4):
                            nc.tensor.matmul(
                                ps,
                                lhsT=gw1c_sb[:, r, gm * 128:(gm + 1) * 128],
                                rhs=cacc[:, r, csl],
                                start=(r == 0),
                                stop=(r == 3),
                            )
                        g1c_ch = p34.tile([128, 512], BF16, tag="g1cch",
                                          bufs=4)
                        nc.vector.tensor_copy(g1c_ch, ps)
                        nc.sync.dma_start(
                            out=rs_in_c[ic][gm * 128:(gm + 1) * 128, :],
                            in_=g1c_ch,
                        )
                    nc.gpsimd.collective_compute(
                        "ReduceScatter",
                        mybir.AluOpType.add,
                        replica_groups=groups,
                        ins=[rs_in_c[ic].opt()],
                        outs=[rs_out_c[ic].opt()],
                    )
                # Pass B: per-chunk gelu chain; all loads/adds on gpsimd so
                # the sync-DMA queue and PE never wait on a collective.
                for ic in range(IC):
                    csl = slice(ic * 512, (ic + 1) * 512)
                    nc.gpsimd.dma_start(out=g1c_sb[:, csl], in_=rs_out_c[ic])
                    gsum = p34.tile([128, 512], F32, tag="gsum", bufs=2)
                    nc.gpsimd.tensor_add(gsum, g1x_sb[:, csl], g1c_sb[:, csl])
                    gt_ch = p34.tile([128, 512], BF16, tag="gt", bufs=2)
                    nc.scalar.activation(gt_ch, gsum, GELU_FUNC,
                                         bias=gb1_sb, scale=1.0)
                    nc.gpsimd.dma_start(out=ag2_in_c[ic], in_=gt_ch)
                    nc.gpsimd.collective_compute(
                        "AllGather",
                        mybir.AluOpType.bypass,
                        replica_groups=groups,
                        ins=[ag2_in_c[ic].opt()],
                        outs=[ag2_out_c[ic].opt()],
                    )
                    gtf = p34.tile([128, NC_, 512], BF16, tag="gtf", bufs=4,
                                   name=f"gtf{ic}")
                    nc.scalar.dma_start(
                        out=gtf,
                        in_=ag2_out_c[ic].rearrange("(r p) i -> p r i", p=128),
                    )
                    gtf_tiles.append(gtf)
                # Pass 2: logits + sigmoid + gated output per i-chunk.
                for ic in range(IC):
                    csl = slice(ic * 512, (ic + 1) * 512)
                    gtf = gtf_tiles[ic]
                    for m in range(4):
                        ps = psum.tile([128, 512], F32, tag="mm", bufs=3,
                                       name="ps_gw2")
                        for r in range(NC_):
                            nc.tensor.matmul(
                                ps,
                                lhsT=gw2_sb[:, r, m * 128:(m + 1) * 128],
                                rhs=gtf[:, r, :],
                                start=(r == 0),
                                stop=(r == NC_ - 1),
                            )
                        gate_ch = p34.tile([128, 512], BF16, tag="gate",
                                           bufs=2)
                        nc.scalar.activation(
                            gate_ch, ps,
                            mybir.ActivationFunctionType.Sigmoid,
                            bias=gb2_sb[:, m:m + 1], scale=1.0,
                        )
                        outt = p34.tile([128, 512], F32, tag="outt", bufs=2)
                        nc.vector.tensor_mul(outt, gate_ch, cacc[:, m, csl])
                        nc.sync.dma_start(
                            out=out_d[m * 128:(m + 1) * 128, csl], in_=outt
                        )

    nc.compile()
    return nc


def _make_in_maps(inputs):
    f32 = np.float32
    bf = ml_dtypes.bfloat16
    f8 = ml_dtypes.float8_e4m3
    X = np.asarray(inputs["hidden_states"], dtype=f32)
    mask = np.asarray(inputs["attention_mask"])
    Wq = np.asarray(inputs["Wq"], dtype=f32)
    Wk = np.asarray(inputs["Wk"], dtype=f32)
    Wv = np.asarray(inputs["Wv"], dtype=f32)
    Wo = np.asarray(inputs["Wo"], dtype=f32)
    gW1 = np.asarray(inputs["gW1"], dtype=f32)
    gb1 = np.asarray(inputs["gb1"], dtype=f32)
    gW2 = np.asarray(inputs["gW2"], dtype=f32)
    gb2 = np.asarray(inputs["gb2"], dtype=f32)

    XT = np.ascontiguousarray(X.T)                       # [4096, 2048]
    XT_bf = XT.astype(bf)
    # Wo row permutation to match per-head AllGather chunk assembly:
    # OT_full row (t*1024 + r*128 + d) holds global head (4r+t), dim d.
    perm = np.empty(HID, dtype=np.int64)
    for t in range(HPC):
        for r in range(NC_):
            g = 4 * r + t
            perm[t * 1024 + r * 128:t * 1024 + (r + 1) * 128] = np.arange(
                g * 128, (g + 1) * 128
            )
    Wo_p = Wo[perm]
    maskb = np.where(mask, 0.0, -1e30).astype(f32)       # [2048]
    maskb_t = np.ascontiguousarray(maskb.reshape(JT, 128).T)  # [128, 16]
    diagm = (1.0 - np.eye(128, dtype=f32)).astype(bf)

    in_maps = []
    for c in range(NC_):
        hsl = slice(c * HS, (c + 1) * HS)
        gsl = slice(c * GS, (c + 1) * GS)
        in_maps.append({
            "xt_bf": XT_bf,
            "wq": np.ascontiguousarray(Wq[:, hsl].astype(bf)),
            "wk": np.ascontiguousarray(Wk[:, hsl].astype(bf)),
            "wv": np.ascontiguousarray(Wv[:, hsl].astype(bf)),
            "wo": np.ascontiguousarray((Wo_p[:, hsl] * WO_SCALE).astype(f8)),
            "gw1x": np.ascontiguousarray(gW1[:HID, gsl].astype(bf)),
            "gw1c": np.ascontiguousarray(
                gW1[HID + c * HS:HID + (c + 1) * HS].astype(bf)),
            "gw2": np.ascontiguousarray(gW2[:, hsl].astype(bf)),
            "gb1": np.ascontiguousarray(gb1[gsl].reshape(GS, 1)),
            "gb2": np.ascontiguousarray(gb2[hsl].reshape(4, 128).T),
            "maskb": maskb_t,
            "diagm": diagm,
        })
    return in_maps


_NC_CACHE = None


def _run(inputs, trace=False):
    global _NC_CACHE
    if _NC_CACHE is None:
        _NC_CACHE = _build_program()
    nc = _NC_CACHE
    in_maps = _make_in_maps(inputs)
    res = bass_utils.run_bass_kernel_spmd(
        nc, in_maps, core_ids=list(range(NC_)), trace=trace
    )
    shards = [np.asarray(res.results[c]["out"], dtype=np.float32)
              for c in range(NC_)]
    gated = np.concatenate(shards, axis=0).T  # gate * cross, [2048, 4096]
    out = np.asarray(inputs["hidden_states"], dtype=np.float32) + gated
    return np.ascontiguousarray(out), res


def kernel(**inputs) -> np.ndarray:
    out, _ = _run(inputs, trace=False)
    return out



# revision 2
# speedup vs baseline: 1.5724x; 1.5724x over previous
"""CrossBatchAttention Trainium2 kernel — 8-core tensor-parallel SPMD.

Layout strategy: every on-chip tensor is kept in transposed [feature, batch]
layout so the TensorEngine contraction dim is always on partitions and no
on-chip transposes are needed. Host numpy does all transposes / casts /
shard slicing, and adds the residual hidden_states at the end.

v2: all large matmuls run fp8 with MatmulPerfMode.DoubleRow (2 k-tiles per
instruction, ~1.5x PE throughput). Weights are pre-scaled by 64 into
fp8e4m3 range; X/V/P stream in fp8. Attention probabilities use fp8e5m2
(wide dynamic range: exp(s - 4ln2) never overflows and the tail never
flushes to zero); the 2^-4 prescale cancels in the softmax normalization.

Per core c (of 8):
  phase 1: QT/KT [512,2048] bf16 (64x scaled), V [2048,512] fp8 (4 local
           heads), g1X (gate W1 X-part) — weights resident, X^T streamed
           in batch-quarters, all matmuls fp8-DoubleRow.
  phase 2: per (head, batch-quarter): S^T = K^T@Q^T per j-tile (bf16),
           ACT Exp((SCALE/4096)*s + bias) -> fp8e5m2, diagonal zeroed,
           denominator via all-ones DoubleRow matmul, O^T = V@P^T
           (DoubleRow), normalize with reciprocal_approx_fast.
           AllGather O^T per head.
  phase 3: cross^T[hid-shard] = Wo[:, shard]^T @ OT_full (column-parallel,
           DoubleRow), k-grouped by AG chunk; last group i-chunk-major and
           feeds the gate chain per chunk.
  phase 4 (pipelined per i-chunk inside phase 3's last group):
           g1C partial (DoubleRow) -> ReduceScatter(gh) -> gelu(fp8) ->
           AllGather(g^T) -> logits (DoubleRow) -> sigmoid -> out^T.
Host: concat 8 [512,2048] shards, transpose, add X -> [2048,4096] f32.
"""

import numpy as np
import ml_dtypes

import concourse.bass as bass
import concourse.mybir as mybir
import concourse.tile as tile
from concourse import bacc
from concourse import bass_utils

BF16 = mybir.dt.bfloat16
F32 = mybir.dt.float32
F8 = mybir.dt.float8e4
F8E5 = mybir.dt.float8e5
DR = mybir.MatmulPerfMode.DoubleRow
W_SCALE = 64.0

B = 2048
HID = 4096
NH = 32
HD = 128
GH = 1024
NC_ = 8
HPC = NH // NC_          # heads per core = 4
HS = HID // NC_          # hid shard = 512
GS = GH // NC_           # gate-hidden shard = 128
SCALE = 1.0 / float(np.sqrt(HD))
EXP_SHIFT = 4 * float(np.log(2.0))   # exp(s - 4ln2): cancels in softmax

KT_TILES = HID // 128    # 32 k-tiles over the 4096 contraction
JT = B // 128            # 16 j-tiles over keys
IC = B // 512            # 4 i-chunks of 512 over batch

GELU_FUNC = mybir.ActivationFunctionType.Gelu


def _build_program():
    nc = bacc.Bacc(
        "TRN2",
        target_bir_lowering=False,
        debug=False,
        enable_asserts=False,
        num_devices=NC_,
    )

    # ---- I/O declarations (per-core shapes) ----
    xt_f8 = nc.dram_tensor("xt_f8", [HID, B], F8, kind="ExternalInput").ap()
    wq_d = nc.dram_tensor("wq", [HID, HS], F8, kind="ExternalInput").ap()
    wk_d = nc.dram_tensor("wk", [HID, HS], F8, kind="ExternalInput").ap()
    wv_d = nc.dram_tensor("wv", [HID, HS], F8, kind="ExternalInput").ap()
    wo_d = nc.dram_tensor("wo", [HID, HS], F8, kind="ExternalInput").ap()
    gw1x_d = nc.dram_tensor("gw1x", [HID, GS], F8, kind="ExternalInput").ap()
    gw1c_d = nc.dram_tensor("gw1c", [HS, GH], F8, kind="ExternalInput").ap()
    gw2_d = nc.dram_tensor("gw2", [GH, HS], F8, kind="ExternalInput").ap()
    gb1_d = nc.dram_tensor("gb1", [GS, 1], F32, kind="ExternalInput").ap()
    gb2_d = nc.dram_tensor("gb2", [128, 4], F32, kind="ExternalInput").ap()
    maskb_d = nc.dram_tensor("maskb", [128, JT], F32, kind="ExternalInput").ap()
    diagm_d = nc.dram_tensor("diagm", [128, 128], F8E5, kind="ExternalInput").ap()
    out_d = nc.dram_tensor("out", [HS, B], F32, kind="ExternalOutput").ap()

    groups = [list(range(NC_))]

    with tile.TileContext(nc) as tc:
        with (
            tc.tile_pool(name="persist", bufs=1) as persist,
            tc.tile_pool(name="psum", bufs=1, space="PSUM") as psum,
            tc.tile_pool(name="dram", bufs=1, space="DRAM") as dram,
        ):
            # ---------- persistent SBUF ----------
            qt_sb = persist.tile([128, HPC, B], BF16)     # [d, head, i] 64x
            kt_sb = persist.tile([128, HPC, B], BF16)     # 64x scaled
            v_sb = persist.tile([128, JT, HS], F8)        # [j_in, j_tile, hd]
            g1x_sb = persist.tile([128, B], F32)          # gate W1 X-part
            maskb_sb = persist.tile([128, JT], F32)
            diagm_sb = persist.tile([128, 128], F8E5)
            ones2_sb = persist.tile([128, 2, 128], F8)
            gb1_sb = persist.tile([GS, 1], F32)
            gb2_sb = persist.tile([128, 4], F32)

            nc.sync.dma_start(out=maskb_sb, in_=maskb_d)
            nc.sync.dma_start(out=diagm_sb, in_=diagm_d)
            nc.sync.dma_start(out=gb1_sb, in_=gb1_d)
            nc.sync.dma_start(out=gb2_sb, in_=gb2_d)
            nc.vector.memset(ones2_sb, 1.0)

            # ---------- DRAM bounce buffers for collectives ----------
            ag_in = dram.tile([HPC, 2, 128, B // 2], F8)
            ag_out = [[None, None] for _ in range(HPC)]
            for h in range(HPC):
                for hf in range(2):
                    t_ag = dram.tile(
                        [NC_ * 128, B // 2], F8, addr_space="Shared",
                        name=f"ag_out{h}_{hf}"
                    )
                    ag_out[h][hf] = t_ag
            rs_in_c, rs_out_c, ag2_in_c, ag2_out_c = [], [], [], []
            for icc in range(IC):
                t_ri = dram.tile([GH, 512], BF16, name=f"rs_in{icc}")
                t_ro = dram.tile([GS, 512], BF16, name=f"rs_out{icc}")
                t_ai = dram.tile([GS, 512], F8, name=f"ag2_in{icc}")
                t_ao = dram.tile([GH, 512], F8, addr_space="Shared",
                                 name=f"ag2_out{icc}")
                rs_in_c.append(t_ri)
                rs_out_c.append(t_ro)
                ag2_in_c.append(t_ai)
                ag2_out_c.append(t_ao)

            warm_rs_i = dram.tile([GH, 64], BF16)
            warm_rs_o = dram.tile([GS, 64], BF16)
            warm_ag_i = dram.tile([GS, 64], BF16)
            warm_ag_o = dram.tile([GH, 64], BF16, addr_space="Shared")
            nc.gpsimd.collective_compute(
                "ReduceScatter", mybir.AluOpType.add, replica_groups=groups,
                ins=[warm_rs_i.opt()], outs=[warm_rs_o.opt()],
            )
            nc.gpsimd.collective_compute(
                "AllGather", mybir.AluOpType.bypass, replica_groups=groups,
                ins=[warm_ag_i.opt()], outs=[warm_ag_o.opt()],
            )

            # =====================================================
            # Phase 1: projections, weights resident, X streamed
            # =====================================================
            with tc.tile_pool(name="p1", bufs=1) as p1:
                wq_sb = p1.tile([128, KT_TILES, HS], F8, tag="wq", bufs=1)
                wk_sb = p1.tile([128, KT_TILES, HS], F8, tag="wk", bufs=1)
                wv_sb = p1.tile([128, KT_TILES, HS], F8, tag="wv", bufs=1)
                gw1x_sb = p1.tile([128, KT_TILES, GS], F8, tag="gw1x", bufs=1)

                def load_w(dst, src, ncols):
                    for hh in range(4):
                        nc.sync.dma_start(
                            out=dst[:, hh * 8:(hh + 1) * 8, :],
                            in_=src[hh * 1024:(hh + 1) * 1024, :].rearrange(
                                "(t p) m -> p t m", p=128
                            ),
                        )

                load_w(wq_sb, wq_d, HS)
                xt_tiles = []
                for q in range(IC):
                    xt_q = p1.tile([128, KT_TILES, 512], F8, tag="xt", bufs=2)
                    xt_tiles.append(xt_q)
                isl0 = slice(0, 512)
                for kk in range(4):
                    nc.sync.dma_start(
                        out=xt_tiles[0][:, kk * 8:(kk + 1) * 8, :],
                        in_=xt_f8[kk * 1024:(kk + 1) * 1024, isl0].rearrange(
                            "(t p) i -> p t i", p=128
                        ),
                    )
                load_w(wk_sb, wk_d, HS)
                load_w(wv_sb, wv_d, HS)
                load_w(gw1x_sb, gw1x_d, GS)

                for q in range(IC):  # 4 quarters of 512 batch elems
                    isl = slice(q * 512, (q + 1) * 512)
                    xt_q = xt_tiles[q]
                    if q + 1 < IC:
                        nxt = slice((q + 1) * 512, (q + 2) * 512)
                        for kk in range(4):
                            nc.sync.dma_start(
                                out=xt_tiles[q + 1][:, kk * 8:(kk + 1) * 8, :],
                                in_=xt_f8[kk * 1024:(kk + 1) * 1024,
                                          nxt].rearrange(
                                    "(t p) i -> p t i", p=128
                                ),
                            )

                    for wsb, dst in ((wq_sb, qt_sb), (wk_sb, kt_sb)):
                        for m in range(4):
                            ps = psum.tile([128, 512], F32, tag="mm", bufs=3,
                                           name="ps_pr")
                            for k in range(KT_TILES // 2):
                                nc.tensor.matmul(
                                    ps,
                                    lhsT=wsb[:, 2 * k:2 * k + 2,
                                             m * 128:(m + 1) * 128],
                                    rhs=xt_q[:, 2 * k:2 * k + 2, :],
                                    start=(k == 0),
                                    stop=(k == KT_TILES // 2 - 1),
                                    perf_mode=DR,
                                )
                            nc.vector.tensor_copy(dst[:, m, isl], ps)
                    # V in natural [j, d] layout: lhsT = X^T tiles
                    for it in range(4):  # 4 i-tiles of 128 in this quarter
                        ps = psum.tile([128, 512], F32, tag="mm", bufs=3,
                                       name="ps_v")
                        for k in range(KT_TILES // 2):
                            nc.tensor.matmul(
                                ps,
                                lhsT=xt_q[:, 2 * k:2 * k + 2,
                                          it * 128:(it + 1) * 128],
                                rhs=wv_sb[:, 2 * k:2 * k + 2, :],
                                start=(k == 0),
                                stop=(k == KT_TILES // 2 - 1),
                                perf_mode=DR,
                            )
                        nc.vector.tensor_scalar_mul(
                            v_sb[:, q * 4 + it, :], ps, 1.0 / W_SCALE
                        )
                    # gate W1 X-part (gh-shard output)
                    ps = psum.tile([128, 512], F32, tag="mm", bufs=3,
                                   name="ps_g1x")
                    for k in range(KT_TILES // 2):
                        nc.tensor.matmul(
                            ps,
                            lhsT=gw1x_sb[:, 2 * k:2 * k + 2, :],
                            rhs=xt_q[:, 2 * k:2 * k + 2, :],
                            start=(k == 0),
                            stop=(k == KT_TILES // 2 - 1),
                            perf_mode=DR,
                        )
                    nc.vector.tensor_scalar_mul(
                        g1x_sb[:, isl], ps, 1.0 / W_SCALE
                    )

            # =====================================================
            # Phase 2: attention per (head, batch-quarter)
            # =====================================================
            with tc.tile_pool(name="p2", bufs=1) as p2:
                for h in range(HPC):
                    for q in range(IC):
                        qsl = slice(q * 512, (q + 1) * 512)
                        den_ps = psum.tile([128, 512], F32, tag="den", bufs=2)
                        ot_ps = psum.tile([128, 512], F32, tag="ot", bufs=2)
                        pt = p2.tile([128, JT, 512], F8E5, tag="pt", bufs=2)
                        for j in range(JT):
                            st = psum.tile([128, 512], F32, tag="mm", bufs=3,
                                           name="st")
                            nc.tensor.matmul(
                                st,
                                lhsT=kt_sb[:, h, j * 128:(j + 1) * 128],
                                rhs=qt_sb[:, h, qsl],
                                start=True,
                                stop=True,
                            )
                            # qt/kt are 64x: fold 1/4096 into the exp scale
                            nc.scalar.activation(
                                pt[:, j, :],
                                st,
                                mybir.ActivationFunctionType.Exp,
                                bias=maskb_sb[:, j:j + 1],
                                scale=SCALE / (W_SCALE * W_SCALE),
                            )
                            # zero the self-attention diagonal block
                            if j // 4 == q:
                                c0 = (j % 4) * 128
                                nc.vector.tensor_mul(
                                    pt[:, j, c0:c0 + 128],
                                    pt[:, j, c0:c0 + 128],
                                    diagm_sb,
                                )
                        for jj in range(JT // 2):
                            nc.tensor.matmul(
                                den_ps,
                                lhsT=ones2_sb,
                                rhs=pt[:, 2 * jj:2 * jj + 2, :],
                                start=(jj == 0),
                                stop=(jj == JT // 2 - 1),
                                perf_mode=DR,
                            )
                            nc.tensor.matmul(
                                ot_ps,
                                lhsT=v_sb[:, 2 * jj:2 * jj + 2,
                                          h * 128:(h + 1) * 128],
                                rhs=pt[:, 2 * jj:2 * jj + 2, :],
                                start=(jj == 0),
                                stop=(jj == JT // 2 - 1),
                                perf_mode=DR,
                            )
                        rec = p2.tile([128, 512], F32, tag="rec", bufs=2)
                        nc.vector.reciprocal_approx_fast(out=rec, in_=den_ps)
                        otc = p2.tile([128, 512], F8, tag="otc", bufs=2)
                        nc.vector.tensor_mul(otc, ot_ps, rec)
                        nc.sync.dma_start(
                            out=ag_in[h, q // 2, :, (q % 2) * 512:
                                      (q % 2) * 512 + 512],
                            in_=otc,
                        )
                        if q % 2 == 1:
                            hf = q // 2
                            nc.gpsimd.collective_compute(
                                "AllGather",
                                mybir.AluOpType.bypass,
                                replica_groups=groups,
                                ins=[ag_in[h, hf].opt()],
                                outs=[ag_out[h][hf].opt()],
                            )

            # =====================================================
            # Phase 3 + 4: out_proj (k-grouped by AG chunk); the last
            # group is i-chunk-major and drives the gate-MLP pipeline
            # =====================================================
            with tc.tile_pool(name="p34", bufs=1) as p34:
                cacc = p34.tile([128, 4, B], BF16, tag="cacc", bufs=1)
                wo_sb = p34.tile([128, KT_TILES, HS], F8, tag="wo", bufs=1)
                nc.sync.dma_start(
                    out=wo_sb, in_=wo_d.rearrange("(t p) m -> p t m", p=128)
                )
                gw1c_sb = p34.tile([128, 4, GH], F8, tag="gw1c", bufs=1)
                nc.sync.dma_start(
                    out=gw1c_sb, in_=gw1c_d.rearrange("(t p) m -> p t m", p=128)
                )
                gw2_sb = p34.tile([128, NC_, HS], F8, tag="gw2", bufs=1)
                nc.sync.dma_start(
                    out=gw2_sb, in_=gw2_d.rearrange("(t p) m -> p t m", p=128)
                )
                g1c_sb = p34.tile([128, B], BF16, tag="g1c", bufs=1)

                def outproj_group(t, ic):
                    csl = slice(ic * 512, (ic + 1) * 512)
                    otg = p34.tile([128, NC_, 512], F8, tag="otg", bufs=4,
                                   name="otg")
                    nc.sync.dma_start(
                        out=otg,
                        in_=ag_out[t][ic // 2][:, (ic % 2) * 512:
                                               (ic % 2) * 512 + 512].rearrange(
                            "(r p) i -> p r i", p=128
                        ),
                    )
                    for m in range(4):
                        ps = psum.tile([128, 512], F32, tag="mm", bufs=3,
                                       name="ps_wo")
                        for r in range(NC_ // 2):
                            nc.tensor.matmul(
                                ps,
                                lhsT=wo_sb[:, t * NC_ + 2 * r:
                                           t * NC_ + 2 * r + 2,
                                           m * 128:(m + 1) * 128],
                                rhs=otg[:, 2 * r:2 * r + 2, :],
                                start=(r == 0),
                                stop=(r == NC_ // 2 - 1),
                                perf_mode=DR,
                            )
                        if t == 0:
                            nc.vector.tensor_scalar_mul(
                                cacc[:, m, csl], ps, 1.0 / W_SCALE
                            )
                        else:
                            nc.vector.scalar_tensor_tensor(
                                cacc[:, m, csl], ps, 1.0 / W_SCALE,
                                cacc[:, m, csl],
                                op0=mybir.AluOpType.mult,
                                op1=mybir.AluOpType.add,
                            )

                for t in range(HPC - 1):
                    for ic in range(IC):
                        outproj_group(t, ic)

                # ---- last k-group, i-chunk-major, feeding the gate chain.
                gtf_tiles = []
                for ic in range(IC):
                    csl = slice(ic * 512, (ic + 1) * 512)
                    outproj_group(HPC - 1, ic)
                    # fp8 copy of this cross^T chunk for the DR g1C matmul
                    cf8 = p34.tile([128, 4, 512], F8, tag="cf8", bufs=2)
                    for m in range(4):
                        nc.vector.tensor_copy(cf8[:, m, :], cacc[:, m, csl])
                    for gm in range(NC_):  # 8 gh-tiles of g1C partial
                        ps = psum.tile([128, 512], F32, tag="mm", bufs=3,
                                       name="ps_g1c")
                        for r in range(2):
                            nc.tensor.matmul(
                                ps,
                                lhsT=gw1c_sb[:, 2 * r:2 * r + 2,
                                             gm * 128:(gm + 1) * 128],
                                rhs=cf8[:, 2 * r:2 * r + 2, :],
                                start=(r == 0),
                                stop=(r == 1),
                                perf_mode=DR,
                            )
                        g1c_ch = p34.tile([128, 512], BF16, tag="g1cch",
                                          bufs=4)
                        nc.vector.tensor_scalar_mul(g1c_ch, ps, 1.0 / W_SCALE)
                        nc.sync.dma_start(
                            out=rs_in_c[ic][gm * 128:(gm + 1) * 128, :],
                            in_=g1c_ch,
                        )
                    nc.gpsimd.collective_compute(
                        "ReduceScatter",
                        mybir.AluOpType.add,
                        replica_groups=groups,
                        ins=[rs_in_c[ic].opt()],
                        outs=[rs_out_c[ic].opt()],
                    )
                # Pass B: per-chunk gelu chain; all loads/adds on gpsimd so
                # the sync-DMA queue and PE never wait on a collective.
                for ic in range(IC):
                    csl = slice(ic * 512, (ic + 1) * 512)
                    nc.gpsimd.dma_start(out=g1c_sb[:, csl], in_=rs_out_c[ic])
                    gsum = p34.tile([128, 512], F32, tag="gsum", bufs=2)
                    nc.gpsimd.tensor_add(gsum, g1x_sb[:, csl], g1c_sb[:, csl])
                    gt_ch = p34.tile([128, 512], F8, tag="gt", bufs=2)
                    nc.scalar.activation(gt_ch, gsum, GELU_FUNC,
                                         bias=gb1_sb, scale=1.0)
                    nc.gpsimd.dma_start(out=ag2_in_c[ic], in_=gt_ch)
                    nc.gpsimd.collective_compute(
                        "AllGather",
                        mybir.AluOpType.bypass,
                        replica_groups=groups,
                        ins=[ag2_in_c[ic].opt()],
                        outs=[ag2_out_c[ic].opt()],
                    )
                    gtf = p34.tile([128, NC_, 512], F8, tag="gtf", bufs=4,
                                   name=f"gtf{ic}")
                    nc.scalar.dma_start(
                        out=gtf,
                        in_=ag2_out_c[ic].rearrange("(r p) i -> p r i", p=128),
                    )
                    gtf_tiles.append(gtf)
                # Pass 2: logits + sigmoid + gated output per i-chunk.
                for ic in range(IC):
                    csl = slice(ic * 512, (ic + 1) * 512)
                    gtf = gtf_tiles[ic]
                    for m in range(4):
                        ps = psum.tile([128, 512], F32, tag="mm", bufs=3,
                                       name="ps_gw2")
                        for r in range(NC_ // 2):
                            nc.tensor.matmul(
                                ps,
                                lhsT=gw2_sb[:, 2 * r:2 * r + 2,
                                            m * 128:(m + 1) * 128],
                                rhs=gtf[:, 2 * r:2 * r + 2, :],
                                start=(r == 0),
                                stop=(r == NC_ // 2 - 1),
                                perf_mode=DR,
                            )
                        gate_ch = p34.tile([128, 512], BF16, tag="gate",
                                           bufs=2)
                        nc.scalar.activation(
                            gate_ch, ps,
                            mybir.ActivationFunctionType.Sigmoid,
                            bias=gb2_sb[:, m:m + 1], scale=1.0 / W_SCALE,
                        )
                        outt = p34.tile([128, 512], F32, tag="outt", bufs=2)
                        nc.vector.tensor_mul(outt, gate_ch, cacc[:, m, csl])
                        nc.sync.dma_start(
                            out=out_d[m * 128:(m + 1) * 128, csl], in_=outt
                        )

    nc.compile()
    return nc


def _make_in_maps(inputs):
    f32 = np.float32
    f8 = ml_dtypes.float8_e4m3
    f8e5 = ml_dtypes.float8_e5m2
    X = np.asarray(inputs["hidden_states"], dtype=f32)
    mask = np.asarray(inputs["attention_mask"])
    Wq = np.asarray(inputs["Wq"], dtype=f32)
    Wk = np.asarray(inputs["Wk"], dtype=f32)
    Wv = np.asarray(inputs["Wv"], dtype=f32)
    Wo = np.asarray(inputs["Wo"], dtype=f32)
    gW1 = np.asarray(inputs["gW1"], dtype=f32)
    gb1 = np.asarray(inputs["gb1"], dtype=f32)
    gW2 = np.asarray(inputs["gW2"], dtype=f32)
    gb2 = np.asarray(inputs["gb2"], dtype=f32)

    XT = np.ascontiguousarray(X.T)                       # [4096, 2048]
    XT_f8 = XT.astype(f8)
    # Wo row permutation to match per-head AllGather chunk assembly:
    # OT_full row (t*1024 + r*128 + d) holds global head (4r+t), dim d.
    perm = np.empty(HID, dtype=np.int64)
    for t in range(HPC):
        for r in range(NC_):
            g = 4 * r + t
            perm[t * 1024 + r * 128:t * 1024 + (r + 1) * 128] = np.arange(
                g * 128, (g + 1) * 128
            )
    Wo_p = Wo[perm]
    # bias: -EXP_SHIFT for valid keys (prescales exp by 2^-4, cancels in
    # normalization), -1e30 for masked keys
    maskb = np.where(mask, -EXP_SHIFT, -1e30).astype(f32)    # [2048]
    maskb_t = np.ascontiguousarray(maskb.reshape(JT, 128).T)  # [128, 16]
    diagm = (1.0 - np.eye(128, dtype=f32)).astype(f8e5)

    in_maps = []
    for c in range(NC_):
        hsl = slice(c * HS, (c + 1) * HS)
        gsl = slice(c * GS, (c + 1) * GS)
        in_maps.append({
            "xt_f8": XT_f8,
            "wq": np.ascontiguousarray((Wq[:, hsl] * W_SCALE).astype(f8)),
            "wk": np.ascontiguousarray((Wk[:, hsl] * W_SCALE).astype(f8)),
            "wv": np.ascontiguousarray((Wv[:, hsl] * W_SCALE).astype(f8)),
            "wo": np.ascontiguousarray((Wo_p[:, hsl] * W_SCALE).astype(f8)),
            "gw1x": np.ascontiguousarray(
                (gW1[:HID, gsl] * W_SCALE).astype(f8)),
            "gw1c": np.ascontiguousarray(
                (gW1[HID + c * HS:HID + (c + 1) * HS] * W_SCALE).astype(f8)),
            "gw2": np.ascontiguousarray((gW2[:, hsl] * W_SCALE).astype(f8)),
            "gb1": np.ascontiguousarray(gb1[gsl].reshape(GS, 1)),
            "gb2": np.ascontiguousarray(gb2[hsl].reshape(4, 128).T),
            "maskb": maskb_t,
            "diagm": diagm,
        })
    return in_maps


_NC_CACHE = None


def _run(inputs, trace=False):
    global _NC_CACHE
    if _NC_CACHE is None:
        _NC_CACHE = _build_program()
    nc = _NC_CACHE
    in_maps = _make_in_maps(inputs)
    res = bass_utils.run_bass_kernel_spmd(
        nc, in_maps, core_ids=list(range(NC_)), trace=trace
    )
    shards = [np.asarray(res.results[c]["out"], dtype=np.float32)
              for c in range(NC_)]
    gated = np.concatenate(shards, axis=0).T  # gate * cross, [2048, 4096]
    out = np.asarray(inputs["hidden_states"], dtype=np.float32) + gated
    return np.ascontiguousarray(out), res


def kernel(**inputs) -> np.ndarray:
    out, _ = _run(inputs, trace=False)
    return out


# revision 5
# speedup vs baseline: 1.7094x; 1.0871x over previous
"""CrossBatchAttention Trainium2 kernel — 8-core tensor-parallel SPMD.

Layout strategy: every on-chip tensor is kept in transposed [feature, batch]
layout so the TensorEngine contraction dim is always on partitions and no
on-chip transposes are needed. Host numpy does all transposes / casts /
shard slicing, and adds the residual hidden_states at the end.

v2: all large matmuls run fp8 with MatmulPerfMode.DoubleRow (2 k-tiles per
instruction, ~1.5x PE throughput). Weights are pre-scaled by 64 into
fp8e4m3 range; X/V/P stream in fp8. Attention probabilities use fp8e5m2
(wide dynamic range: exp(s - 4ln2) never overflows and the tail never
flushes to zero); the 2^-4 prescale cancels in the softmax normalization.

Per core c (of 8):
  phase 1: QT/KT [512,2048] bf16 (64x scaled), V [2048,512] fp8 (4 local
           heads), g1X (gate W1 X-part) — weights resident, X^T streamed
           in batch-quarters, all matmuls fp8-DoubleRow.
  phase 2: per (head, batch-quarter): S^T = K^T@Q^T per j-tile (bf16),
           ACT Exp((SCALE/4096)*s + bias) -> fp8e5m2, diagonal zeroed,
           denominator via all-ones DoubleRow matmul, O^T = V@P^T
           (DoubleRow), normalize with reciprocal_approx_fast.
           AllGather O^T per head.
  phase 3: cross^T[hid-shard] = Wo[:, shard]^T @ OT_full (column-parallel,
           DoubleRow), k-grouped by AG chunk; last group i-chunk-major and
           feeds the gate chain per chunk.
  phase 4 (pipelined per i-chunk inside phase 3's last group):
           g1C partial (DoubleRow) -> ReduceScatter(gh) -> gelu(fp8) ->
           AllGather(g^T) -> logits (DoubleRow) -> sigmoid -> out^T.
Host: concat 8 [512,2048] shards, transpose, add X -> [2048,4096] f32.
"""

import numpy as np
import ml_dtypes

import concourse.bass as bass
import concourse.mybir as mybir
import concourse.tile as tile
from concourse import bacc
from concourse import bass_utils

BF16 = mybir.dt.bfloat16
F32 = mybir.dt.float32
F8 = mybir.dt.float8e4
F8E5 = mybir.dt.float8e5
DR = mybir.MatmulPerfMode.DoubleRow
W_SCALE = 64.0

B = 2048
HID = 4096
NH = 32
HD = 128
GH = 1024
NC_ = 8
HPC = NH // NC_          # heads per core = 4
HS = HID // NC_          # hid shard = 512
GS = GH // NC_           # gate-hidden shard = 128
SCALE = 1.0 / float(np.sqrt(HD))
EXP_SHIFT = 4 * float(np.log(2.0))   # exp(s - 4ln2): cancels in softmax

KT_TILES = HID // 128    # 32 k-tiles over the 4096 contraction
JT = B // 128            # 16 j-tiles over keys
IC = B // 512            # 4 i-chunks of 512 over batch

GELU_FUNC = mybir.ActivationFunctionType.Gelu


def _build_program():
    nc = bacc.Bacc(
        "TRN2",
        target_bir_lowering=False,
        debug=False,
        enable_asserts=False,
        num_devices=NC_,
    )

    # ---- I/O declarations (per-core shapes) ----
    xt_f8 = nc.dram_tensor("xt_f8", [HID, B], F8, kind="ExternalInput").ap()
    wq_d = nc.dram_tensor("wq", [HID, HS], F8, kind="ExternalInput").ap()
    wk_d = nc.dram_tensor("wk", [HID, HS], F8, kind="ExternalInput").ap()
    wv_d = nc.dram_tensor("wv", [HID, HS], F8, kind="ExternalInput").ap()
    wo_d = nc.dram_tensor("wo", [HID, HS], F8, kind="ExternalInput").ap()
    gw1x_d = nc.dram_tensor("gw1x", [HID, GS], F8, kind="ExternalInput").ap()
    gw1c_d = nc.dram_tensor("gw1c", [HS, GH], F8, kind="ExternalInput").ap()
    gw2_d = nc.dram_tensor("gw2", [GH, HS], F8, kind="ExternalInput").ap()
    gb1_d = nc.dram_tensor("gb1", [GS, 1], F32, kind="ExternalInput").ap()
    gb2_d = nc.dram_tensor("gb2", [128, 4], F32, kind="ExternalInput").ap()
    maskb_d = nc.dram_tensor("maskb", [128, JT], F32, kind="ExternalInput").ap()
    diagm_d = nc.dram_tensor("diagm", [128, 128], F8E5, kind="ExternalInput").ap()
    out_d = nc.dram_tensor("out", [HS, B], BF16, kind="ExternalOutput").ap()

    groups = [list(range(NC_))]

    with tile.TileContext(nc) as tc:
        with (
            tc.tile_pool(name="persist", bufs=1) as persist,
            tc.tile_pool(name="psum", bufs=1, space="PSUM") as psum,
            tc.tile_pool(name="dram", bufs=1, space="DRAM") as dram,
        ):
            # ---------- persistent SBUF ----------
            qt_sb = persist.tile([128, HPC, B], BF16)     # [d, head, i] 64x
            kt_sb = persist.tile([128, HPC, B], BF16)     # 64x scaled
            v_sb = persist.tile([128, JT, HS], F8)        # [j_in, j_tile, hd]
            g1x_sb = persist.tile([128, B], F32)          # gate W1 X-part
            maskb_sb = persist.tile([128, JT], F32)
            diagm_sb = persist.tile([128, 128], F8E5)
            ones2_sb = persist.tile([128, 2, 128], F8)
            gb1_sb = persist.tile([GS, 1], F32)
            gb2_sb = persist.tile([128, 4], F32)

            nc.sync.dma_start(out=maskb_sb, in_=maskb_d)
            nc.sync.dma_start(out=diagm_sb, in_=diagm_d)
            nc.sync.dma_start(out=gb1_sb, in_=gb1_d)
            nc.sync.dma_start(out=gb2_sb, in_=gb2_d)
            nc.vector.memset(ones2_sb, 1.0)

            # ---------- DRAM bounce buffers for collectives ----------
            ag_in = dram.tile([HPC, 2, 128, B // 2], F8)
            ag_out = [[None, None] for _ in range(HPC)]
            for h in range(HPC):
                for hf in range(2):
                    t_ag = dram.tile(
                        [NC_ * 128, B // 2], F8, addr_space="Shared",
                        name=f"ag_out{h}_{hf}"
                    )
                    ag_out[h][hf] = t_ag
            rs_in_c, rs_out_c, ag2_in_c, ag2_out_c = [], [], [], []
            for icc in range(IC):
                t_ri = dram.tile([GH, 512], BF16, name=f"rs_in{icc}")
                t_ro = dram.tile([GS, 512], BF16, name=f"rs_out{icc}")
                t_ai = dram.tile([GS, 512], F8, name=f"ag2_in{icc}")
                t_ao = dram.tile([GH, 512], F8, addr_space="Shared",
                                 name=f"ag2_out{icc}")
                rs_in_c.append(t_ri)
                rs_out_c.append(t_ro)
                ag2_in_c.append(t_ai)
                ag2_out_c.append(t_ao)

            warm_rs_i = dram.tile([GH, 64], BF16)
            warm_rs_o = dram.tile([GS, 64], BF16)
            warm_ag_i = dram.tile([GS, 64], BF16)
            warm_ag_o = dram.tile([GH, 64], BF16, addr_space="Shared")
            nc.gpsimd.collective_compute(
                "ReduceScatter", mybir.AluOpType.add, replica_groups=groups,
                ins=[warm_rs_i.opt()], outs=[warm_rs_o.opt()],
            )
            nc.gpsimd.collective_compute(
                "AllGather", mybir.AluOpType.bypass, replica_groups=groups,
                ins=[warm_ag_i.opt()], outs=[warm_ag_o.opt()],
            )

            # =====================================================
            # Phase 1: projections, weights resident, X streamed
            # =====================================================
            with tc.tile_pool(name="p1", bufs=1) as p1:
                wq_sb = p1.tile([128, KT_TILES, HS], F8, tag="wq", bufs=1)
                wk_sb = p1.tile([128, KT_TILES, HS], F8, tag="wk", bufs=1)
                wv_sb = p1.tile([128, KT_TILES, HS], F8, tag="wv", bufs=1)
                gw1x_sb = p1.tile([128, KT_TILES, GS], F8, tag="gw1x", bufs=1)

                def load_w(dst, src, ncols):
                    for hh in range(4):
                        nc.sync.dma_start(
                            out=dst[:, hh * 8:(hh + 1) * 8, :],
                            in_=src[hh * 1024:(hh + 1) * 1024, :].rearrange(
                                "(t p) m -> p t m", p=128
                            ),
                        )

                load_w(wq_sb, wq_d, HS)
                xt_tiles = []
                for q in range(IC):
                    xt_q = p1.tile([128, KT_TILES, 512], F8, tag="xt", bufs=2)
                    xt_tiles.append(xt_q)
                isl0 = slice(0, 512)
                for kk in range(4):
                    nc.sync.dma_start(
                        out=xt_tiles[0][:, kk * 8:(kk + 1) * 8, :],
                        in_=xt_f8[kk * 1024:(kk + 1) * 1024, isl0].rearrange(
                            "(t p) i -> p t i", p=128
                        ),
                    )
                load_w(wk_sb, wk_d, HS)
                load_w(wv_sb, wv_d, HS)
                load_w(gw1x_sb, gw1x_d, GS)

                for q in range(IC):  # 4 quarters of 512 batch elems
                    isl = slice(q * 512, (q + 1) * 512)
                    xt_q = xt_tiles[q]
                    if q + 1 < IC:
                        nxt = slice((q + 1) * 512, (q + 2) * 512)
                        for kk in range(4):
                            nc.sync.dma_start(
                                out=xt_tiles[q + 1][:, kk * 8:(kk + 1) * 8, :],
                                in_=xt_f8[kk * 1024:(kk + 1) * 1024,
                                          nxt].rearrange(
                                    "(t p) i -> p t i", p=128
                                ),
                            )

                    for wsb, dst in ((wq_sb, qt_sb), (wk_sb, kt_sb)):
                        for m in range(4):
                            ps = psum.tile([128, 512], F32, tag="mm", bufs=3,
                                           name="ps_pr")
                            for k in range(KT_TILES // 2):
                                nc.tensor.matmul(
                                    ps,
                                    lhsT=wsb[:, 2 * k:2 * k + 2,
                                             m * 128:(m + 1) * 128],
                                    rhs=xt_q[:, 2 * k:2 * k + 2, :],
                                    start=(k == 0),
                                    stop=(k == KT_TILES // 2 - 1),
                                    perf_mode=DR,
                                )
                            nc.vector.tensor_copy(dst[:, m, isl], ps)
                    # V in natural [j, d] layout: lhsT = X^T tiles
                    for it in range(4):  # 4 i-tiles of 128 in this quarter
                        ps = psum.tile([128, 512], F32, tag="mm", bufs=3,
                                       name="ps_v")
                        for k in range(KT_TILES // 2):
                            nc.tensor.matmul(
                                ps,
                                lhsT=xt_q[:, 2 * k:2 * k + 2,
                                          it * 128:(it + 1) * 128],
                                rhs=wv_sb[:, 2 * k:2 * k + 2, :],
                                start=(k == 0),
                                stop=(k == KT_TILES // 2 - 1),
                                perf_mode=DR,
                            )
                        nc.vector.tensor_scalar_mul(
                            v_sb[:, q * 4 + it, :], ps, 1.0 / W_SCALE
                        )
                    # gate W1 X-part (gh-shard output)
                    ps = psum.tile([128, 512], F32, tag="mm", bufs=3,
                                   name="ps_g1x")
                    for k in range(KT_TILES // 2):
                        nc.tensor.matmul(
                            ps,
                            lhsT=gw1x_sb[:, 2 * k:2 * k + 2, :],
                            rhs=xt_q[:, 2 * k:2 * k + 2, :],
                            start=(k == 0),
                            stop=(k == KT_TILES // 2 - 1),
                            perf_mode=DR,
                        )
                    nc.vector.tensor_scalar_mul(
                        g1x_sb[:, isl], ps, 1.0 / W_SCALE
                    )

            # =====================================================
            # Phase 2: attention per (head, batch-quarter).
            # Loop order (q-half outer, head inner) spreads the O^T
            # AllGathers across the phase so phase 3 never waits.
            # =====================================================
            with tc.tile_pool(name="p2", bufs=1) as p2:
                for qh in range(2):
                  for h in range(HPC):
                    for q in (2 * qh, 2 * qh + 1):
                        qsl = slice(q * 512, (q + 1) * 512)
                        den_ps = psum.tile([128, 512], F32, tag="den", bufs=2)
                        ot_ps = psum.tile([128, 512], F32, tag="ot", bufs=2)
                        pt = p2.tile([128, JT, 512], F8E5, tag="pt", bufs=2)
                        for j in range(JT):
                            st = psum.tile([128, 512], F32, tag="mm", bufs=3,
                                           name="st")
                            nc.tensor.matmul(
                                st,
                                lhsT=kt_sb[:, h, j * 128:(j + 1) * 128],
                                rhs=qt_sb[:, h, qsl],
                                start=True,
                                stop=True,
                            )
                            # qt/kt are 64x: fold 1/4096 into the exp scale
                            nc.scalar.activation(
                                pt[:, j, :],
                                st,
                                mybir.ActivationFunctionType.Exp,
                                bias=maskb_sb[:, j:j + 1],
                                scale=SCALE / (W_SCALE * W_SCALE),
                            )
                            # zero the self-attention diagonal block
                            if j // 4 == q:
                                c0 = (j % 4) * 128
                                nc.vector.tensor_mul(
                                    pt[:, j, c0:c0 + 128],
                                    pt[:, j, c0:c0 + 128],
                                    diagm_sb,
                                )
                        for jj in range(JT // 2):
                            nc.tensor.matmul(
                                den_ps,
                                lhsT=ones2_sb,
                                rhs=pt[:, 2 * jj:2 * jj + 2, :],
                                start=(jj == 0),
                                stop=(jj == JT // 2 - 1),
                                perf_mode=DR,
                            )
                            nc.tensor.matmul(
                                ot_ps,
                                lhsT=v_sb[:, 2 * jj:2 * jj + 2,
                                          h * 128:(h + 1) * 128],
                                rhs=pt[:, 2 * jj:2 * jj + 2, :],
                                start=(jj == 0),
                                stop=(jj == JT // 2 - 1),
                                perf_mode=DR,
                            )
                        rec = p2.tile([128, 512], F32, tag="rec", bufs=2)
                        nc.vector.reciprocal_approx_fast(out=rec, in_=den_ps)
                        otc = p2.tile([128, 512], F8, tag="otc", bufs=2)
                        nc.vector.tensor_mul(otc, ot_ps, rec)
                        nc.sync.dma_start(
                            out=ag_in[h, q // 2, :, (q % 2) * 512:
                                      (q % 2) * 512 + 512],
                            in_=otc,
                        )
                        if q % 2 == 1:
                            hf = q // 2
                            nc.gpsimd.collective_compute(
                                "AllGather",
                                mybir.AluOpType.bypass,
                                replica_groups=groups,
                                ins=[ag_in[h, hf].opt()],
                                outs=[ag_out[h][hf].opt()],
                            )

            # =====================================================
            # Phase 3 + 4: out_proj (k-grouped by AG chunk); the last
            # group is i-chunk-major and drives the gate-MLP pipeline
            # =====================================================
            with tc.tile_pool(name="p34", bufs=1) as p34:
                cacc = p34.tile([128, 4, B], BF16, tag="cacc", bufs=1)
                wo_sb = p34.tile([128, KT_TILES, HS], F8, tag="wo", bufs=1)
                nc.sync.dma_start(
                    out=wo_sb, in_=wo_d.rearrange("(t p) m -> p t m", p=128)
                )
                gw1c_sb = p34.tile([128, 4, GH], F8, tag="gw1c", bufs=1)
                nc.sync.dma_start(
                    out=gw1c_sb, in_=gw1c_d.rearrange("(t p) m -> p t m", p=128)
                )
                gw2_sb = p34.tile([128, NC_, HS], F8, tag="gw2", bufs=1)
                nc.sync.dma_start(
                    out=gw2_sb, in_=gw2_d.rearrange("(t p) m -> p t m", p=128)
                )
                g1c_sb = p34.tile([128, B], BF16, tag="g1c", bufs=1)

                def outproj_group(t, ic):
                    csl = slice(ic * 512, (ic + 1) * 512)
                    otg = p34.tile([128, NC_, 512], F8, tag="otg", bufs=4,
                                   name="otg")
                    nc.sync.dma_start(
                        out=otg,
                        in_=ag_out[t][ic // 2][:, (ic % 2) * 512:
                                               (ic % 2) * 512 + 512].rearrange(
                            "(r p) i -> p r i", p=128
                        ),
                    )
                    for m in range(4):
                        ps = psum.tile([128, 512], F32, tag="mm", bufs=3,
                                       name="ps_wo")
                        for r in range(NC_ // 2):
                            nc.tensor.matmul(
                                ps,
                                lhsT=wo_sb[:, t * NC_ + 2 * r:
                                           t * NC_ + 2 * r + 2,
                                           m * 128:(m + 1) * 128],
                                rhs=otg[:, 2 * r:2 * r + 2, :],
                                start=(r == 0),
                                stop=(r == NC_ // 2 - 1),
                                perf_mode=DR,
                            )
                        if t == 0:
                            nc.vector.tensor_scalar_mul(
                                cacc[:, m, csl], ps, 1.0 / W_SCALE
                            )
                        else:
                            nc.vector.scalar_tensor_tensor(
                                cacc[:, m, csl], ps, 1.0 / W_SCALE,
                                cacc[:, m, csl],
                                op0=mybir.AluOpType.mult,
                                op1=mybir.AluOpType.add,
                            )

                # ---- i-chunk-major: each chunk's full out_proj (all 4
                # k-groups) completes early so its ReduceScatter overlaps
                # the next chunk's PE work.
                gtf_tiles = []
                for ic in range(IC):
                    csl = slice(ic * 512, (ic + 1) * 512)
                    for t in range(HPC):
                        outproj_group(t, ic)
                    # fp8 copy of this cross^T chunk for the DR g1C matmul
                    cf8 = p34.tile([128, 4, 512], F8, tag="cf8", bufs=2)
                    for m in range(4):
                        nc.vector.tensor_copy(cf8[:, m, :], cacc[:, m, csl])
                    for gm in range(NC_):  # 8 gh-tiles of g1C partial
                        ps = psum.tile([128, 512], F32, tag="mm", bufs=3,
                                       name="ps_g1c")
                        for r in range(2):
                            nc.tensor.matmul(
                                ps,
                                lhsT=gw1c_sb[:, 2 * r:2 * r + 2,
                                             gm * 128:(gm + 1) * 128],
                                rhs=cf8[:, 2 * r:2 * r + 2, :],
                                start=(r == 0),
                                stop=(r == 1),
                                perf_mode=DR,
                            )
                        g1c_ch = p34.tile([128, 512], BF16, tag="g1cch",
                                          bufs=4)
                        nc.vector.tensor_scalar_mul(g1c_ch, ps, 1.0 / W_SCALE)
                        nc.sync.dma_start(
                            out=rs_in_c[ic][gm * 128:(gm + 1) * 128, :],
                            in_=g1c_ch,
                        )
                    nc.gpsimd.collective_compute(
                        "ReduceScatter",
                        mybir.AluOpType.add,
                        replica_groups=groups,
                        ins=[rs_in_c[ic].opt()],
                        outs=[rs_out_c[ic].opt()],
                    )
                # Pass B: per-chunk gelu chain; all loads/adds on gpsimd so
                # the sync-DMA queue and PE never wait on a collective.
                for ic in range(IC):
                    csl = slice(ic * 512, (ic + 1) * 512)
                    nc.gpsimd.dma_start(out=g1c_sb[:, csl], in_=rs_out_c[ic])
                    gsum = p34.tile([128, 512], F32, tag="gsum", bufs=2)
                    nc.gpsimd.tensor_add(gsum, g1x_sb[:, csl], g1c_sb[:, csl])
                    gt_ch = p34.tile([128, 512], F8, tag="gt", bufs=2)
                    nc.scalar.activation(gt_ch, gsum, GELU_FUNC,
                                         bias=gb1_sb, scale=1.0)
                    nc.gpsimd.dma_start(out=ag2_in_c[ic], in_=gt_ch)
                    nc.gpsimd.collective_compute(
                        "AllGather",
                        mybir.AluOpType.bypass,
                        replica_groups=groups,
                        ins=[ag2_in_c[ic].opt()],
                        outs=[ag2_out_c[ic].opt()],
                    )
                    gtf = p34.tile([128, NC_, 512], F8, tag="gtf", bufs=4,
                                   name=f"gtf{ic}")
                    nc.scalar.dma_start(
                        out=gtf,
                        in_=ag2_out_c[ic].rearrange("(r p) i -> p r i", p=128),
                    )
                    gtf_tiles.append(gtf)
                # Pass 2: logits + sigmoid + gated output per i-chunk.
                for ic in range(IC):
                    csl = slice(ic * 512, (ic + 1) * 512)
                    gtf = gtf_tiles[ic]
                    for m in range(4):
                        ps = psum.tile([128, 512], F32, tag="mm", bufs=3,
                                       name="ps_gw2")
                        for r in range(NC_ // 2):
                            nc.tensor.matmul(
                                ps,
                                lhsT=gw2_sb[:, 2 * r:2 * r + 2,
                                            m * 128:(m + 1) * 128],
                                rhs=gtf[:, 2 * r:2 * r + 2, :],
                                start=(r == 0),
                                stop=(r == NC_ // 2 - 1),
                                perf_mode=DR,
                            )
                        gate_ch = p34.tile([128, 512], BF16, tag="gate",
                                           bufs=2)
                        nc.scalar.activation(
                            gate_ch, ps,
                            mybir.ActivationFunctionType.Sigmoid,
                            bias=gb2_sb[:, m:m + 1], scale=1.0 / W_SCALE,
                        )
                        outt = p34.tile([128, 512], BF16, tag="outt",
                                        bufs=2)
                        nc.vector.tensor_mul(outt, gate_ch, cacc[:, m, csl])
                        nc.sync.dma_start(
                            out=out_d[m * 128:(m + 1) * 128, csl], in_=outt
                        )

    nc.compile()
    return nc


def _make_in_maps(inputs):
    f32 = np.float32
    f8 = ml_dtypes.float8_e4m3
    f8e5 = ml_dtypes.float8_e5m2
    X = np.asarray(inputs["hidden_states"], dtype=f32)
    mask = np.asarray(inputs["attention_mask"])
    Wq = np.asarray(inputs["Wq"], dtype=f32)
    Wk = np.asarray(inputs["Wk"], dtype=f32)
    Wv = np.asarray(inputs["Wv"], dtype=f32)
    Wo = np.asarray(inputs["Wo"], dtype=f32)
    gW1 = np.asarray(inputs["gW1"], dtype=f32)
    gb1 = np.asarray(inputs["gb1"], dtype=f32)
    gW2 = np.asarray(inputs["gW2"], dtype=f32)
    gb2 = np.asarray(inputs["gb2"], dtype=f32)

    XT = np.ascontiguousarray(X.T)                       # [4096, 2048]
    XT_f8 = XT.astype(f8)
    # Wo row permutation to match per-head AllGather chunk assembly:
    # OT_full row (t*1024 + r*128 + d) holds global head (4r+t), dim d.
    perm = np.empty(HID, dtype=np.int64)
    for t in range(HPC):
        for r in range(NC_):
            g = 4 * r + t
            perm[t * 1024 + r * 128:t * 1024 + (r + 1) * 128] = np.arange(
                g * 128, (g + 1) * 128
            )
    Wo_p = Wo[perm]
    # bias: -EXP_SHIFT for valid keys (prescales exp by 2^-4, cancels in
    # normalization), -1e30 for masked keys
    maskb = np.where(mask, -EXP_SHIFT, -1e30).astype(f32)    # [2048]
    maskb_t = np.ascontiguousarray(maskb.reshape(JT, 128).T)  # [128, 16]
    diagm = (1.0 - np.eye(128, dtype=f32)).astype(f8e5)

    in_maps = []
    for c in range(NC_):
        hsl = slice(c * HS, (c + 1) * HS)
        gsl = slice(c * GS, (c + 1) * GS)
        in_maps.append({
            "xt_f8": XT_f8,
            "wq": np.ascontiguousarray((Wq[:, hsl] * W_SCALE).astype(f8)),
            "wk": np.ascontiguousarray((Wk[:, hsl] * W_SCALE).astype(f8)),
            "wv": np.ascontiguousarray((Wv[:, hsl] * W_SCALE).astype(f8)),
            "wo": np.ascontiguousarray((Wo_p[:, hsl] * W_SCALE).astype(f8)),
            "gw1x": np.ascontiguousarray(
                (gW1[:HID, gsl] * W_SCALE).astype(f8)),
            "gw1c": np.ascontiguousarray(
                (gW1[HID + c * HS:HID + (c + 1) * HS] * W_SCALE).astype(f8)),
            "gw2": np.ascontiguousarray((gW2[:, hsl] * W_SCALE).astype(f8)),
            "gb1": np.ascontiguousarray(gb1[gsl].reshape(GS, 1)),
            "gb2": np.ascontiguousarray(gb2[hsl].reshape(4, 128).T),
            "maskb": maskb_t,
            "diagm": diagm,
        })
    return in_maps


_NC_CACHE = None


def _run(inputs, trace=False):
    global _NC_CACHE
    if _NC_CACHE is None:
        _NC_CACHE = _build_program()
    nc = _NC_CACHE
    in_maps = _make_in_maps(inputs)
    res = bass_utils.run_bass_kernel_spmd(
        nc, in_maps, core_ids=list(range(NC_)), trace=trace
    )
    shards = [np.asarray(res.results[c]["out"]).astype(np.float32)
              for c in range(NC_)]
    gated = np.concatenate(shards, axis=0).T  # gate * cross, [2048, 4096]
    out = np.asarray(inputs["hidden_states"], dtype=np.float32) + gated
    return np.ascontiguousarray(out), res


def kernel(**inputs) -> np.ndarray:
    out, _ = _run(inputs, trace=False)
    return out


# revision 6
# speedup vs baseline: 1.7530x; 1.0255x over previous
"""CrossBatchAttention Trainium2 kernel — 8-core tensor-parallel SPMD.

Layout strategy: every on-chip tensor is kept in transposed [feature, batch]
layout so the TensorEngine contraction dim is always on partitions and no
on-chip transposes are needed. Host numpy does all transposes / casts /
shard slicing, and adds the residual hidden_states at the end.

v2: all large matmuls run fp8 with MatmulPerfMode.DoubleRow (2 k-tiles per
instruction, ~1.5x PE throughput). Weights are pre-scaled by 64 into
fp8e4m3 range; X/V/P stream in fp8. Attention probabilities use fp8e5m2
(wide dynamic range: exp(s - 4ln2) never overflows and the tail never
flushes to zero); the 2^-4 prescale cancels in the softmax normalization.

Per core c (of 8):
  phase 1: QT/KT [512,2048] bf16 (64x scaled), V [2048,512] fp8 (4 local
           heads), g1X (gate W1 X-part) — weights resident, X^T streamed
           in batch-quarters, all matmuls fp8-DoubleRow.
  phase 2: per (head, batch-quarter): S^T = K^T@Q^T per j-tile (bf16),
           ACT Exp((SCALE/4096)*s + bias) -> fp8e5m2, diagonal zeroed,
           denominator via all-ones DoubleRow matmul, O^T = V@P^T
           (DoubleRow), normalize with reciprocal_approx_fast.
           AllGather O^T per head.
  phase 3: cross^T[hid-shard] = Wo[:, shard]^T @ OT_full (column-parallel,
           DoubleRow), k-grouped by AG chunk; last group i-chunk-major and
           feeds the gate chain per chunk.
  phase 4 (pipelined per i-chunk inside phase 3's last group):
           g1C partial (DoubleRow) -> ReduceScatter(gh) -> gelu(fp8) ->
           AllGather(g^T) -> logits (DoubleRow) -> sigmoid -> out^T.
Host: concat 8 [512,2048] shards, transpose, add X -> [2048,4096] f32.
"""

import numpy as np
import ml_dtypes

import concourse.bass as bass
import concourse.mybir as mybir
import concourse.tile as tile
from concourse import bacc
from concourse import bass_utils

BF16 = mybir.dt.bfloat16
F32 = mybir.dt.float32
F8 = mybir.dt.float8e4
F8E5 = mybir.dt.float8e5
DR = mybir.MatmulPerfMode.DoubleRow
W_SCALE = 64.0

B = 2048
HID = 4096
NH = 32
HD = 128
GH = 1024
NC_ = 8
HPC = NH // NC_          # heads per core = 4
HS = HID // NC_          # hid shard = 512
GS = GH // NC_           # gate-hidden shard = 128
SCALE = 1.0 / float(np.sqrt(HD))
EXP_SHIFT = 4 * float(np.log(2.0))   # exp(s - 4ln2): cancels in softmax

KT_TILES = HID // 128    # 32 k-tiles over the 4096 contraction
JT = B // 128            # 16 j-tiles over keys
IC = B // 512            # 4 i-chunks of 512 over batch

GELU_FUNC = mybir.ActivationFunctionType.Gelu


def _build_program():
    nc = bacc.Bacc(
        "TRN2",
        target_bir_lowering=False,
        debug=False,
        enable_asserts=False,
        num_devices=NC_,
    )

    # ---- I/O declarations (per-core shapes) ----
    xt_f8 = nc.dram_tensor("xt_f8", [HID, B], F8, kind="ExternalInput").ap()
    wq_d = nc.dram_tensor("wq", [HID, HS], F8, kind="ExternalInput").ap()
    wk_d = nc.dram_tensor("wk", [HID, HS], F8, kind="ExternalInput").ap()
    wv_d = nc.dram_tensor("wv", [HID, HS], F8, kind="ExternalInput").ap()
    wo_d = nc.dram_tensor("wo", [HID, HS], F8, kind="ExternalInput").ap()
    gw1x_d = nc.dram_tensor("gw1x", [HID, GS], F8, kind="ExternalInput").ap()
    gw1c_d = nc.dram_tensor("gw1c", [HS, GH], F8, kind="ExternalInput").ap()
    gw2_d = nc.dram_tensor("gw2", [GH, HS], F8, kind="ExternalInput").ap()
    gb1_d = nc.dram_tensor("gb1", [GS, 1], F32, kind="ExternalInput").ap()
    gb2_d = nc.dram_tensor("gb2", [128, 4], F32, kind="ExternalInput").ap()
    maskb_d = nc.dram_tensor("maskb", [128, JT], F32, kind="ExternalInput").ap()
    diagm_d = nc.dram_tensor("diagm", [128, 128], F8E5, kind="ExternalInput").ap()
    out_d = nc.dram_tensor("out", [HS, B], BF16, kind="ExternalOutput").ap()

    groups = [list(range(NC_))]

    with tile.TileContext(nc) as tc:
        with (
            tc.tile_pool(name="persist", bufs=1) as persist,
            tc.tile_pool(name="psum", bufs=1, space="PSUM") as psum,
            tc.tile_pool(name="dram", bufs=1, space="DRAM") as dram,
        ):
            # ---------- persistent SBUF ----------
            qt_sb = persist.tile([128, HPC, B], BF16)     # [d, head, i] 64x
            kt_sb = persist.tile([128, HPC, B], BF16)     # 64x scaled
            v_sb = persist.tile([128, JT, HS], F8)        # [j_in, j_tile, hd]
            g1x_sb = persist.tile([128, B], F32)          # gate W1 X-part
            maskb_sb = persist.tile([128, JT], F32)
            diagm_sb = persist.tile([128, 128], F8E5)
            ones2_sb = persist.tile([128, 2, 128], F8)
            gb1_sb = persist.tile([GS, 1], F32)
            gb2_sb = persist.tile([128, 4], F32)

            nc.sync.dma_start(out=maskb_sb, in_=maskb_d)
            nc.sync.dma_start(out=diagm_sb, in_=diagm_d)
            nc.sync.dma_start(out=gb1_sb, in_=gb1_d)
            nc.sync.dma_start(out=gb2_sb, in_=gb2_d)
            nc.vector.memset(ones2_sb, 1.0)

            # ---------- DRAM bounce buffers for collectives ----------
            ag_in = dram.tile([HPC, 2, 128, B // 2], F8)
            ag_out = [[None, None] for _ in range(HPC)]
            for h in range(HPC):
                for hf in range(2):
                    t_ag = dram.tile(
                        [NC_ * 128, B // 2], F8, addr_space="Shared",
                        name=f"ag_out{h}_{hf}"
                    )
                    ag_out[h][hf] = t_ag
            rs_in_c, rs_out_c, ag2_in_c, ag2_out_c = [], [], [], []
            for icc in range(IC):
                t_ri = dram.tile([GH, 512], BF16, name=f"rs_in{icc}")
                t_ro = dram.tile([GS, 512], BF16, name=f"rs_out{icc}")
                t_ai = dram.tile([GS, 512], F8, name=f"ag2_in{icc}")
                t_ao = dram.tile([GH, 512], F8, addr_space="Shared",
                                 name=f"ag2_out{icc}")
                rs_in_c.append(t_ri)
                rs_out_c.append(t_ro)
                ag2_in_c.append(t_ai)
                ag2_out_c.append(t_ao)

            warm_rs_i = dram.tile([GH, 64], BF16)
            warm_rs_o = dram.tile([GS, 64], BF16)
            warm_ag_i = dram.tile([GS, 64], BF16)
            warm_ag_o = dram.tile([GH, 64], BF16, addr_space="Shared")
            nc.gpsimd.collective_compute(
                "ReduceScatter", mybir.AluOpType.add, replica_groups=groups,
                ins=[warm_rs_i.opt()], outs=[warm_rs_o.opt()],
            )
            nc.gpsimd.collective_compute(
                "AllGather", mybir.AluOpType.bypass, replica_groups=groups,
                ins=[warm_ag_i.opt()], outs=[warm_ag_o.opt()],
            )

            # =====================================================
            # Phase 1: projections, weights resident, X streamed
            # =====================================================
            with tc.tile_pool(name="p1", bufs=1) as p1:
                wq_sb = p1.tile([128, KT_TILES, HS], F8, tag="wq", bufs=1)
                wk_sb = p1.tile([128, KT_TILES, HS], F8, tag="wk", bufs=1)
                wv_sb = p1.tile([128, KT_TILES, HS], F8, tag="wv", bufs=1)
                gw1x_sb = p1.tile([128, KT_TILES, GS], F8, tag="gw1x", bufs=1)

                def load_w(dst, src, ncols):
                    for hh in range(4):
                        nc.sync.dma_start(
                            out=dst[:, hh * 8:(hh + 1) * 8, :],
                            in_=src[hh * 1024:(hh + 1) * 1024, :].rearrange(
                                "(t p) m -> p t m", p=128
                            ),
                        )

                load_w(wq_sb, wq_d, HS)
                xt_tiles = []
                for q in range(IC):
                    xt_q = p1.tile([128, KT_TILES, 512], F8, tag="xt", bufs=2)
                    xt_tiles.append(xt_q)
                isl0 = slice(0, 512)
                for kk in range(4):
                    nc.sync.dma_start(
                        out=xt_tiles[0][:, kk * 8:(kk + 1) * 8, :],
                        in_=xt_f8[kk * 1024:(kk + 1) * 1024, isl0].rearrange(
                            "(t p) i -> p t i", p=128
                        ),
                    )
                load_w(wk_sb, wk_d, HS)
                load_w(wv_sb, wv_d, HS)
                load_w(gw1x_sb, gw1x_d, GS)

                for q in range(IC):  # 4 quarters of 512 batch elems
                    isl = slice(q * 512, (q + 1) * 512)
                    xt_q = xt_tiles[q]
                    if q + 1 < IC:
                        nxt = slice((q + 1) * 512, (q + 2) * 512)
                        for kk in range(4):
                            nc.sync.dma_start(
                                out=xt_tiles[q + 1][:, kk * 8:(kk + 1) * 8, :],
                                in_=xt_f8[kk * 1024:(kk + 1) * 1024,
                                          nxt].rearrange(
                                    "(t p) i -> p t i", p=128
                                ),
                            )

                    for wsb, dst in ((wq_sb, qt_sb), (wk_sb, kt_sb)):
                        for m in range(4):
                            ps = psum.tile([128, 512], F32, tag="mm", bufs=3,
                                           name="ps_pr")
                            for k in range(KT_TILES // 2):
                                nc.tensor.matmul(
                                    ps,
                                    lhsT=wsb[:, 2 * k:2 * k + 2,
                                             m * 128:(m + 1) * 128],
                                    rhs=xt_q[:, 2 * k:2 * k + 2, :],
                                    start=(k == 0),
                                    stop=(k == KT_TILES // 2 - 1),
                                    perf_mode=DR,
                                )
                            nc.vector.tensor_copy(dst[:, m, isl], ps)
                    # V in natural [j, d] layout: lhsT = X^T tiles
                    for it in range(4):  # 4 i-tiles of 128 in this quarter
                        ps = psum.tile([128, 512], F32, tag="mm", bufs=3,
                                       name="ps_v")
                        for k in range(KT_TILES // 2):
                            nc.tensor.matmul(
                                ps,
                                lhsT=xt_q[:, 2 * k:2 * k + 2,
                                          it * 128:(it + 1) * 128],
                                rhs=wv_sb[:, 2 * k:2 * k + 2, :],
                                start=(k == 0),
                                stop=(k == KT_TILES // 2 - 1),
                                perf_mode=DR,
                            )
                        nc.vector.tensor_scalar_mul(
                            v_sb[:, q * 4 + it, :], ps, 1.0 / W_SCALE
                        )
                    # gate W1 X-part (gh-shard output)
                    ps = psum.tile([128, 512], F32, tag="mm", bufs=3,
                                   name="ps_g1x")
                    for k in range(KT_TILES // 2):
                        nc.tensor.matmul(
                            ps,
                            lhsT=gw1x_sb[:, 2 * k:2 * k + 2, :],
                            rhs=xt_q[:, 2 * k:2 * k + 2, :],
                            start=(k == 0),
                            stop=(k == KT_TILES // 2 - 1),
                            perf_mode=DR,
                        )
                    nc.vector.tensor_scalar_mul(
                        g1x_sb[:, isl], ps, 1.0 / W_SCALE
                    )

            # =====================================================
            # Phases 2-4 merged: attention blocks, out_proj chunks and
            # the gate-MLP chain interleaved so collectives overlap PE
            # work. Second-half head order h3..h0 matches the order in
            # which out_proj consumes the hf=1 AllGathers.
            # =====================================================
            with (
                tc.tile_pool(name="p2", bufs=1) as p2,
                tc.tile_pool(name="p34", bufs=1) as p34,
            ):
                # phase-3/4 weights load up front, hidden under phase 2
                cacc = p34.tile([128, 4, B], BF16, tag="cacc", bufs=1)
                wo_sb = p34.tile([128, KT_TILES, HS], F8, tag="wo", bufs=1)
                nc.sync.dma_start(
                    out=wo_sb, in_=wo_d.rearrange("(t p) m -> p t m", p=128)
                )
                gw1c_sb = p34.tile([128, 4, GH], F8, tag="gw1c", bufs=1)
                nc.sync.dma_start(
                    out=gw1c_sb, in_=gw1c_d.rearrange("(t p) m -> p t m", p=128)
                )
                gw2_sb = p34.tile([128, NC_, HS], F8, tag="gw2", bufs=1)
                nc.sync.dma_start(
                    out=gw2_sb, in_=gw2_d.rearrange("(t p) m -> p t m", p=128)
                )
                g1c_sb = p34.tile([128, B], BF16, tag="g1c", bufs=1)
                gtf_tiles = {}

                def attn_block(h, q):
                    qsl = slice(q * 512, (q + 1) * 512)
                    den_ps = psum.tile([128, 512], F32, tag="den", bufs=2)
                    ot_ps = psum.tile([128, 512], F32, tag="ot", bufs=2)
                    pt = p2.tile([128, JT, 512], F8E5, tag="pt", bufs=2)
                    for j in range(JT):
                        st = psum.tile([128, 512], F32, tag="mm", bufs=3,
                                       name="st")
                        nc.tensor.matmul(
                            st,
                            lhsT=kt_sb[:, h, j * 128:(j + 1) * 128],
                            rhs=qt_sb[:, h, qsl],
                            start=True,
                            stop=True,
                        )
                        # qt/kt are 64x: fold 1/4096 into the exp scale
                        nc.scalar.activation(
                            pt[:, j, :],
                            st,
                            mybir.ActivationFunctionType.Exp,
                            bias=maskb_sb[:, j:j + 1],
                            scale=SCALE / (W_SCALE * W_SCALE),
                        )
                        # zero the self-attention diagonal block
                        if j // 4 == q:
                            c0 = (j % 4) * 128
                            nc.vector.tensor_mul(
                                pt[:, j, c0:c0 + 128],
                                pt[:, j, c0:c0 + 128],
                                diagm_sb,
                            )
                    for jj in range(JT // 2):
                        nc.tensor.matmul(
                            den_ps,
                            lhsT=ones2_sb,
                            rhs=pt[:, 2 * jj:2 * jj + 2, :],
                            start=(jj == 0),
                            stop=(jj == JT // 2 - 1),
                            perf_mode=DR,
                        )
                        nc.tensor.matmul(
                            ot_ps,
                            lhsT=v_sb[:, 2 * jj:2 * jj + 2,
                                      h * 128:(h + 1) * 128],
                            rhs=pt[:, 2 * jj:2 * jj + 2, :],
                            start=(jj == 0),
                            stop=(jj == JT // 2 - 1),
                            perf_mode=DR,
                        )
                    rec = p2.tile([128, 512], F32, tag="rec", bufs=2)
                    nc.vector.reciprocal_approx_fast(out=rec, in_=den_ps)
                    otc = p2.tile([128, 512], F8, tag="otc", bufs=2)
                    nc.vector.tensor_mul(otc, ot_ps, rec)
                    nc.sync.dma_start(
                        out=ag_in[h, q // 2, :, (q % 2) * 512:
                                  (q % 2) * 512 + 512],
                        in_=otc,
                    )

                def ag_ot(h, hf):
                    nc.gpsimd.collective_compute(
                        "AllGather",
                        mybir.AluOpType.bypass,
                        replica_groups=groups,
                        ins=[ag_in[h, hf].opt()],
                        outs=[ag_out[h][hf].opt()],
                    )

                def outproj_group(t, ic, init):
                    csl = slice(ic * 512, (ic + 1) * 512)
                    otg = p34.tile([128, NC_, 512], F8, tag="otg", bufs=4,
                                   name="otg")
                    nc.sync.dma_start(
                        out=otg,
                        in_=ag_out[t][ic // 2][:, (ic % 2) * 512:
                                               (ic % 2) * 512 + 512].rearrange(
                            "(r p) i -> p r i", p=128
                        ),
                    )
                    for m in range(4):
                        ps = psum.tile([128, 512], F32, tag="mm", bufs=3,
                                       name="ps_wo")
                        for r in range(NC_ // 2):
                            nc.tensor.matmul(
                                ps,
                                lhsT=wo_sb[:, t * NC_ + 2 * r:
                                           t * NC_ + 2 * r + 2,
                                           m * 128:(m + 1) * 128],
                                rhs=otg[:, 2 * r:2 * r + 2, :],
                                start=(r == 0),
                                stop=(r == NC_ // 2 - 1),
                                perf_mode=DR,
                            )
                        if init:
                            nc.vector.tensor_scalar_mul(
                                cacc[:, m, csl], ps, 1.0 / W_SCALE
                            )
                        else:
                            nc.vector.scalar_tensor_tensor(
                                cacc[:, m, csl], ps, 1.0 / W_SCALE,
                                cacc[:, m, csl],
                                op0=mybir.AluOpType.mult,
                                op1=mybir.AluOpType.add,
                            )

                def gate_g1c(ic):
                    csl = slice(ic * 512, (ic + 1) * 512)
                    # fp8 copy of this cross^T chunk for the DR g1C matmul
                    cf8 = p34.tile([128, 4, 512], F8, tag="cf8", bufs=2)
                    for m in range(4):
                        nc.vector.tensor_copy(cf8[:, m, :], cacc[:, m, csl])
                    for gm in range(NC_):  # 8 gh-tiles of g1C partial
                        ps = psum.tile([128, 512], F32, tag="mm", bufs=3,
                                       name="ps_g1c")
                        for r in range(2):
                            nc.tensor.matmul(
                                ps,
                                lhsT=gw1c_sb[:, 2 * r:2 * r + 2,
                                             gm * 128:(gm + 1) * 128],
                                rhs=cf8[:, 2 * r:2 * r + 2, :],
                                start=(r == 0),
                                stop=(r == 1),
                                perf_mode=DR,
                            )
                        g1c_ch = p34.tile([128, 512], BF16, tag="g1cch",
                                          bufs=4)
                        nc.vector.tensor_scalar_mul(g1c_ch, ps, 1.0 / W_SCALE)
                        nc.sync.dma_start(
                            out=rs_in_c[ic][gm * 128:(gm + 1) * 128, :],
                            in_=g1c_ch,
                        )
                    nc.gpsimd.collective_compute(
                        "ReduceScatter",
                        mybir.AluOpType.add,
                        replica_groups=groups,
                        ins=[rs_in_c[ic].opt()],
                        outs=[rs_out_c[ic].opt()],
                    )

                def pass_b(ic):
                    csl = slice(ic * 512, (ic + 1) * 512)
                    nc.gpsimd.dma_start(out=g1c_sb[:, csl], in_=rs_out_c[ic])
                    gsum = p34.tile([128, 512], F32, tag="gsum", bufs=2)
                    nc.gpsimd.tensor_add(gsum, g1x_sb[:, csl], g1c_sb[:, csl])
                    gt_ch = p34.tile([128, 512], F8, tag="gt", bufs=2)
                    nc.scalar.activation(gt_ch, gsum, GELU_FUNC,
                                         bias=gb1_sb, scale=1.0)
                    nc.gpsimd.dma_start(out=ag2_in_c[ic], in_=gt_ch)
                    nc.gpsimd.collective_compute(
                        "AllGather",
                        mybir.AluOpType.bypass,
                        replica_groups=groups,
                        ins=[ag2_in_c[ic].opt()],
                        outs=[ag2_out_c[ic].opt()],
                    )
                    gtf = p34.tile([128, NC_, 512], F8, tag="gtf", bufs=4,
                                   name=f"gtf{ic}")
                    nc.scalar.dma_start(
                        out=gtf,
                        in_=ag2_out_c[ic].rearrange("(r p) i -> p r i", p=128),
                    )
                    gtf_tiles[ic] = gtf

                def gw2_chunk(ic):
                    csl = slice(ic * 512, (ic + 1) * 512)
                    gtf = gtf_tiles[ic]
                    for m in range(4):
                        ps = psum.tile([128, 512], F32, tag="mm", bufs=3,
                                       name="ps_gw2")
                        for r in range(NC_ // 2):
                            nc.tensor.matmul(
                                ps,
                                lhsT=gw2_sb[:, 2 * r:2 * r + 2,
                                            m * 128:(m + 1) * 128],
                                rhs=gtf[:, 2 * r:2 * r + 2, :],
                                start=(r == 0),
                                stop=(r == NC_ // 2 - 1),
                                perf_mode=DR,
                            )
                        gate_ch = p34.tile([128, 512], BF16, tag="gate",
                                           bufs=2)
                        nc.scalar.activation(
                            gate_ch, ps,
                            mybir.ActivationFunctionType.Sigmoid,
                            bias=gb2_sb[:, m:m + 1], scale=1.0 / W_SCALE,
                        )
                        outt = p34.tile([128, 512], BF16, tag="outt",
                                        bufs=2)
                        nc.vector.tensor_mul(outt, gate_ch, cacc[:, m, csl])
                        nc.sync.dma_start(
                            out=out_d[m * 128:(m + 1) * 128, csl], in_=outt
                        )

                # ---- schedule ----
                # first half: batch quarters 0,1 for all heads
                for h in range(HPC):
                    attn_block(h, 0)
                    attn_block(h, 1)
                    ag_ot(h, 0)
                # second half interleaved with out_proj + gate chain
                attn_block(3, 2)
                attn_block(3, 3)
                ag_ot(3, 1)
                for t in range(HPC):
                    outproj_group(t, 0, init=(t == 0))
                gate_g1c(0)
                attn_block(2, 2)
                attn_block(2, 3)
                ag_ot(2, 1)
                for t in range(HPC):
                    outproj_group(t, 1, init=(t == 0))
                gate_g1c(1)
                attn_block(1, 2)
                attn_block(1, 3)
                ag_ot(1, 1)
                pass_b(0)
                attn_block(0, 2)
                attn_block(0, 3)
                ag_ot(0, 1)
                pass_b(1)
                for t in (3, 2, 1):
                    outproj_group(t, 2, init=(t == 3))
                for t in (3, 2, 1):
                    outproj_group(t, 3, init=(t == 3))
                outproj_group(0, 2, init=False)
                gate_g1c(2)
                outproj_group(0, 3, init=False)
                gate_g1c(3)
                gw2_chunk(0)
                gw2_chunk(1)
                pass_b(2)
                pass_b(3)
                gw2_chunk(2)
                gw2_chunk(3)

    nc.compile()
    return nc


def _make_in_maps(inputs):
    f32 = np.float32
    f8 = ml_dtypes.float8_e4m3
    f8e5 = ml_dtypes.float8_e5m2
    X = np.asarray(inputs["hidden_states"], dtype=f32)
    mask = np.asarray(inputs["attention_mask"])
    Wq = np.asarray(inputs["Wq"], dtype=f32)
    Wk = np.asarray(inputs["Wk"], dtype=f32)
    Wv = np.asarray(inputs["Wv"], dtype=f32)
    Wo = np.asarray(inputs["Wo"], dtype=f32)
    gW1 = np.asarray(inputs["gW1"], dtype=f32)
    gb1 = np.asarray(inputs["gb1"], dtype=f32)
    gW2 = np.asarray(inputs["gW2"], dtype=f32)
    gb2 = np.asarray(inputs["gb2"], dtype=f32)

    XT = np.ascontiguousarray(X.T)                       # [4096, 2048]
    XT_f8 = XT.astype(f8)
    # Wo row permutation to match per-head AllGather chunk assembly:
    # OT_full row (t*1024 + r*128 + d) holds global head (4r+t), dim d.
    perm = np.empty(HID, dtype=np.int64)
    for t in range(HPC):
        for r in range(NC_):
            g = 4 * r + t
            perm[t * 1024 + r * 128:t * 1024 + (r + 1) * 128] = np.arange(
                g * 128, (g + 1) * 128
            )
    Wo_p = Wo[perm]
    # bias: -EXP_SHIFT for valid keys (prescales exp by 2^-4, cancels in
    # normalization), -1e30 for masked keys
    maskb = np.where(mask, -EXP_SHIFT, -1e30).astype(f32)    # [2048]
    maskb_t = np.ascontiguousarray(maskb.reshape(JT, 128).T)  # [128, 16]
    diagm = (1.0 - np.eye(128, dtype=f32)).astype(f8e5)

    in_maps = []
    for c in range(NC_):
        hsl = slice(c * HS, (c + 1) * HS)
        gsl = slice(c * GS, (c + 1) * GS)
        in_maps.append({
            "xt_f8": XT_f8,
            "wq": np.ascontiguousarray((Wq[:, hsl] * W_SCALE).astype(f8)),
            "wk": np.ascontiguousarray((Wk[:, hsl] * W_SCALE).astype(f8)),
            "wv": np.ascontiguousarray((Wv[:, hsl] * W_SCALE).astype(f8)),
            "wo": np.ascontiguousarray((Wo_p[:, hsl] * W_SCALE).astype(f8)),
            "gw1x": np.ascontiguousarray(
                (gW1[:HID, gsl] * W_SCALE).astype(f8)),
            "gw1c": np.ascontiguousarray(
                (gW1[HID + c * HS:HID + (c + 1) * HS] * W_SCALE).astype(f8)),
            "gw2": np.ascontiguousarray((gW2[:, hsl] * W_SCALE).astype(f8)),
            "gb1": np.ascontiguousarray(gb1[gsl].reshape(GS, 1)),
            "gb2": np.ascontiguousarray(gb2[hsl].reshape(4, 128).T),
            "maskb": maskb_t,
            "diagm": diagm,
        })
    return in_maps


_NC_CACHE = None


def _run(inputs, trace=False):
    global _NC_CACHE
    if _NC_CACHE is None:
        _NC_CACHE = _build_program()
    nc = _NC_CACHE
    in_maps = _make_in_maps(inputs)
    res = bass_utils.run_bass_kernel_spmd(
        nc, in_maps, core_ids=list(range(NC_)), trace=trace
    )
    shards = [np.asarray(res.results[c]["out"]).astype(np.float32)
              for c in range(NC_)]
    gated = np.concatenate(shards, axis=0).T  # gate * cross, [2048, 4096]
    out = np.asarray(inputs["hidden_states"], dtype=np.float32) + gated
    return np.ascontiguousarray(out), res


def kernel(**inputs) -> np.ndarray:
    out, _ = _run(inputs, trace=False)
    return out
